# revision 46
# baseline (speedup 1.0000x reference)
"""Online Normalization (forward) on 8 Trainium2 NeuronCores.

Reference semantics (per batch sample t, stats per channel over H*W):
    out_t = (x_t - s_mu_{t-1}) / sqrt(s_var_{t-1} + eps)
    mu_t  = mean(x_t);  var_t = mean(x_t^2) - mu_t^2
    s_mu_t  = a*s_mu_{t-1}  + (1-a)*mu_t
    s_var_t = a*s_var_{t-1} + (1-a)*var_t + a*(1-a)*(mu_t - s_mu_{t-1})^2

The EMA recurrence is linear, so per-sample batch stats feed small
lower-triangular matmuls on the tensor engine:
    s_mu_{t-1}  = a^t mu0  + sum_i W[i,t] mu_i,   W[i,t] = (1-a) a^{t-1-i}, i<t
    s_var_{t-1} = a^t var0 + sum_i W[i,t] f_i,    f_i = var_i + a*d_i^2,
                                                  d_i = mu_i - s_mu_{i-1}
The scan runs incrementally over tapered groups of samples so normalized
output streams out while later samples stream in.

Engine plan (v3): x lives in SBUF/HBM as fp16 (halves DMA traffic; the
correctness gate is 2e-2, fp16 quantization is ~4e-4).
  - DVE streams BN_STATS (mean+M2 per 512-elem block in one pass -- this
    replaces separate sum and square passes) plus a few small per-group
    reductions; nothing else sits in its queue except one tiny reciprocal
    per group, issued one group late so it never stalls the stream.
  - ACT streams all 32 normalizes (Identity w/ per-partition scale+bias)
    plus one small Sqrt per group.
  - Pool (gpsimd) runs the small PSUM<->SBUF copies and f-vector algebra
    of the stats chain, and triggers the output DMAs (SWDGE).
  - PE does the stats matmuls in [t, c] layout: operand-swapped combine
    (no transposes needed until the final [c, t] flip), with the mu0/var0
    init and eps folded in as extra contraction rows.

Sharding: channels C=256 split across 8 cores (32 each). Per core the
8 MiB fp16 shard is [128 partitions, 32 t, 1024 f], partition p = q*32+c
(q = one of 4 spatial blocks, c = channel).
"""

import os
import sys

import numpy as np

sys.path.insert(0, "/opt/trn_rl_repo")

B = 32          # batch (sequential scan axis)
H = 64
W_SP = 64
C = 256
NCORES = 8
CS = C // NCORES    # 32 channels per core
Q = 4               # spatial blocks per sample
F = (H * W_SP) // Q  # 1024 elements per block
P = 128             # partitions (Q*CS)
AFWD = 0.999
EPS = 1e-5
# tapered scan groups (= DMA chunk sizes, in batch samples): small head for
# fast pipeline fill, small tail so the final chain+normalize drains fast
GROUPS = [2, 4, 6, 8, 8, 4]
assert sum(GROUPS) == B

LAST_EXEC_NS = None
LAST_RESULTS = None
_COMPILED = {}


def _ensure_ntff_hook():
    """The axon boot degrades silently when ``antenv.axon_hooks`` is missing;
    provide the module + the ctypes-based NRT-profile hook ourselves so
    ``run_bass_kernel_spmd(trace=True)`` can capture NTFF profiles."""
    try:
        from antenv.axon_hooks import get_axon_ntff_profile_hook  # noqa: F401

        return
    except ImportError:
        pass

    import contextlib
    import ctypes
    import types

    so_path = "/opt/axon/libaxon_pjrt.so"
    state = {"hook": None}

    mod = types.ModuleType("antenv.axon_hooks")

    def set_axon_ntff_profile_hook(h):
        state["hook"] = h

    def get_axon_ntff_profile_hook():
        return state["hook"]

    mod.set_axon_ntff_profile_hook = set_axon_ntff_profile_hook
    mod.get_axon_ntff_profile_hook = get_axon_ntff_profile_hook
    import antenv

    antenv.axon_hooks = mod
    sys.modules["antenv.axon_hooks"] = mod

    if not os.path.exists(so_path):
        return
    lib = ctypes.CDLL(so_path)
    if not hasattr(lib, "axon_start_nrt_profile"):
        return
    lib.axon_start_nrt_profile.argtypes = [
        ctypes.POINTER(ctypes.c_int64),
        ctypes.c_size_t,
    ]
    lib.axon_start_nrt_profile.restype = ctypes.c_int64
    lib.axon_stop_nrt_profile.argtypes = [ctypes.c_char_p]
    lib.axon_stop_nrt_profile.restype = ctypes.c_int64

    @contextlib.contextmanager
    def _hook(output_dir, device_ids):
        import jax

        jax.devices()
        if device_ids:
            ids = (ctypes.c_int64 * len(device_ids))(*device_ids)
            rc = lib.axon_start_nrt_profile(ids, len(device_ids))
        else:
            rc = lib.axon_start_nrt_profile(None, 0)
        if rc != 0:
            raise RuntimeError(f"axon_start_nrt_profile rc={rc}")
        try:
            yield
        finally:
            n = lib.axon_stop_nrt_profile(str(output_dir).encode())
            print(f"profile: {n} file(s) written to {output_dir}", file=sys.stderr)

    state["hook"] = _hook


def _patch_fishpath():
    """The _compat FishPath shim lacks pathlib conveniences the manifest
    capture/replay helpers use."""
    import pathlib

    from concourse import _compat

    def _open(self, mode="r"):
        p = pathlib.Path(str(self))
        if "w" in mode:
            p.parent.mkdir(parents=True, exist_ok=True)
        return open(str(p), mode)

    _compat.FishPath.open = _open
    _compat.FishPath.mkdir = lambda self, **kw: pathlib.Path(str(self)).mkdir(**kw)
    _compat.FishPath.__fspath__ = lambda self: str(self)
    if not hasattr(_compat.FishPath, "parent"):
        _compat.FishPath.parent = property(
            lambda self: _compat.FishPath(pathlib.Path(str(self)).parent)
        )
    if not hasattr(_compat.FishPath, "stem"):
        _compat.FishPath.stem = property(
            lambda self: pathlib.Path(str(self)).stem
        )


def _manifest_capture_main():
    """Subprocess entry: build (schedule-only) under
    TILE_CAPTURE_MANIFEST_PATH so the schedule manifest lands on disk."""
    _patch_fishpath()
    try:
        _build_bass_raw(skip_compile=True)
    except Exception as e:  # manifest is written before trailing debug steps
        print(f"capture pass ended with: {e}", file=sys.stderr)


def _edit_manifest(path):
    """Rewrite the captured schedule order to pure issue order (sort by
    instruction number). The issue order is hand-pipelined so that every
    small cross-engine chain op sits right after the bn_stats group that
    feeds it; the CoreSim list scheduler instead floats those ops ~2 groups
    late, which serializes the whole back half of the kernel."""
    import json
    import re

    with open(path) as f:
        d = json.load(f)
    for block, order in d["order"].items():
        order.sort(key=lambda e: int(re.match(r"I-(\d+)", e["name"]).group(1)))
    with open(path, "w") as f:
        json.dump(d, f)


def _build_bass():
    import glob
    import subprocess
    import tempfile

    mdir = tempfile.mkdtemp(prefix="norm_manifest_")
    here = os.path.dirname(os.path.abspath(__file__))
    env = {**os.environ, "TILE_CAPTURE_MANIFEST_PATH": mdir}
    env.pop("TILE_SCHEDULER", None)
    env.pop("TILE_LOAD_MANIFEST_PATH", None)
    try:
        subprocess.run(
            [
                sys.executable,
                "-c",
                f"import sys; sys.path.insert(0, {here!r}); "
                "import kernel; kernel._manifest_capture_main()",
            ],
            env=env,
            timeout=600,
            check=False,
        )
        manifests = glob.glob(os.path.join(mdir, "*.json"))
        assert len(manifests) == 1, f"expected 1 manifest, got {manifests}"
        _edit_manifest(manifests[0])
        _patch_fishpath()
        os.environ["TILE_SCHEDULER"] = "manifest"
        os.environ["TILE_LOAD_MANIFEST_PATH"] = mdir
        try:
            return _build_bass_raw()
        finally:
            os.environ.pop("TILE_SCHEDULER", None)
            os.environ.pop("TILE_LOAD_MANIFEST_PATH", None)
    except Exception as e:
        print(f"manifest schedule failed ({e}); default scheduler", file=sys.stderr)
        os.environ.pop("TILE_SCHEDULER", None)
        os.environ.pop("TILE_LOAD_MANIFEST_PATH", None)
        return _build_bass_raw()


def _build_bass_raw(skip_compile=False):
    from contextlib import ExitStack

    import concourse.bacc as bacc
    import concourse.tile as tile
    from concourse import mybir

    DT = mybir.dt.float32
    DT16 = mybir.dt.float16
    Alu = mybir.AluOpType
    Act = mybir.ActivationFunctionType
    Ax = mybir.AxisListType

    nc = bacc.Bacc(
        "TRN2", target_bir_lowering=False, debug=False, num_devices=NCORES
    )
    x_h = nc.declare_dram_parameter("x", [P, B, F], DT16, isOutput=False)
    mask_h = nc.declare_dram_parameter("mask", [P, CS], DT, isOutput=False)
    bmask_h = nc.declare_dram_parameter("bmask", [CS, P], DT, isOutput=False)
    bmaskn_h = nc.declare_dram_parameter("bmaskn", [CS, P], DT, isOutput=False)
    inits_h = nc.declare_dram_parameter("inits", [CS, 2], DT, isOutput=False)
    out_h = nc.declare_dram_parameter("out", [P, B, F], DT16, isOutput=True)

    NG = len(GROUPS)
    LMAX = max(GROUPS)

    with tile.TileContext(nc) as tc, ExitStack() as ctx:
        consts = ctx.enter_context(tc.tile_pool(name="consts", bufs=1))
        xpool = ctx.enter_context(tc.tile_pool(name="xp", bufs=1))
        small = ctx.enter_context(tc.tile_pool(name="small", bufs=1))
        gpool = ctx.enter_context(tc.tile_pool(name="gp", bufs=2))
        psum = ctx.enter_context(tc.tile_pool(name="ps", bufs=1, space="PSUM"))

        # one tile per group: per-group input DMAs, bn_stats reads, in-place
        # normalizes, and output DMAs then carry NO false dependencies on
        # other groups' data. Trigger the first two groups' input DMAs ahead
        # of the const loads so the bn_stats stream starts as early as
        # possible; the consts are only needed ~15us in.
        xg = [
            xpool.tile([P, L, F], DT16, tag=f"xg{i}", name=f"xg{i}")
            for i, L in enumerate(GROUPS)
        ]
        xg3 = [t.rearrange("p b (two f) -> p b two f", two=2) for t in xg]
        t0s = []
        t0 = 0
        for L in GROUPS:
            t0s.append(t0)
            t0 += L
        for gi in (0, 1):
            nc.sync.dma_start(
                out=xg[gi], in_=x_h[:, t0s[gi] : t0s[gi] + GROUPS[gi], :]
            )

        sb_mask = consts.tile([P, CS], DT)       # mask[p, c] = [p%CS==c]/16
        nc.sync.dma_start(out=sb_mask, in_=mask_h[:, :])
        sb_bmask = consts.tile([CS, P], DT)      # bmask[c, p] = [p%CS==c]
        nc.sync.dma_start(out=sb_bmask, in_=bmask_h[:, :])
        sb_bmaskn = consts.tile([CS, P], DT)     # -bmask (negates nbias)
        nc.sync.dma_start(out=sb_bmaskn, in_=bmaskn_h[:, :])
        sb_sqrta = consts.tile([CS, B], DT)      # sqrt(AFWD): f = (sqrt(a)d)^2+var
        nc.vector.memset(sb_sqrta, float(AFWD ** 0.5))
        sb_afwd = consts.tile([CS, B], DT)       # scan multiplier a
        nc.vector.memset(sb_afwd, AFWD)
        sb_oma = consts.tile([CS, B], DT)        # 1-a (scales f for the var scan)
        nc.vector.memset(sb_oma, 1.0 - AFWD)
        sb_eps = consts.tile([CS, 1], DT)
        nc.vector.memset(sb_eps, EPS)

        for gi in range(2, len(GROUPS)):
            nc.sync.dma_start(
                out=xg[gi], in_=x_h[:, t0s[gi] : t0s[gi] + GROUPS[gi], :]
            )

        # bn_stats records: per sample 2 blocks x (even, odd) halves
        # = 4 records of (count, mean, M2)
        bnout = small.tile([P, B, 4, 3], DT)
        bnout4 = bnout.rearrange("p b (k two) three -> p b k (two three)", two=2)
        mean2 = small.tile([P, LMAX, 4], DT)
        sm2 = small.tile([P, LMAX], DT)
        sM2 = small.tile([P, LMAX], DT)
        # stats2[:, 0, t] = sum_x/256 per partition-block; [:, 1, t] = sum_x2/256
        stats2 = small.tile([P, 2, B], DT)
        nc.vector.memset(stats2, 0.0)

        # [c, t] layout state. The EMA recurrences run as tensor_tensor_scan
        # along the free (t) axis with fp32 internal state -- exactly the
        # reference recurrence, no W matrices and no transposes. Column 0 of
        # each scan tile holds the initial state (mu0 / var0), so columns
        # 0..B-1 of the tile ARE the "previous" states the outputs need.
        # smu/sc are double-buffered across groups (written in stage_b(g+1)
        # while stage_c(g) still reads them).
        mumsq_ct = small.tile([CS, 2, B], DT)    # raw mu / msq, ct layout
        muls_ct = small.tile([CS, B], DT)        # (1-a) * mu
        smu_sbs, svar_sbs, sc_cts = [], [], []
        for k in range(2):
            t_smu = small.tile([CS, 1 + B], DT, name=f"smu_sb{k}")
            nc.sync.dma_start(out=t_smu[:, 0:1], in_=inits_h[:, 0:1])
            smu_sbs.append(t_smu)
            t_svar = small.tile([CS, 1 + B], DT, name=f"svar_sb{k}")
            nc.sync.dma_start(out=t_svar[:, 0:1], in_=inits_h[:, 1:2])
            svar_sbs.append(t_svar)
            t_sc = small.tile([CS, B], DT, name=f"sc_ct{k}")
            sc_cts.append(t_sc)
        rs_ct = small.tile([CS, B], DT)
        nb_ct = small.tile([CS, B], DT)
        rb = small.tile([P, 2, B], DT)          # [:,0,t]=rscale, [:,1,t]=nbias

        # warm the sqrt_and_others activation table before the streaming
        # phase so no ACT_TABLE_LOAD lands mid-kernel
        warm = small.tile([1, 1], DT)
        nc.vector.memset(warm, 1.0)
        nc.scalar.activation(out=warm, in_=warm, func=Act.Sqrt)

        t0s = []
        t0 = 0
        for L in GROUPS:
            t0s.append(t0)
            t0 += L

        # Three-stage software pipeline with a 1-group lag between stages:
        # every small DVE op (PSUM copies, reciprocal) gets a full group of
        # bn_stats issued ahead of it, so its cross-engine producers are
        # long done when the DVE queue reaches it -- the bn_stats stream
        # never stalls. GPSIMD cannot touch PSUM, so PSUM->SBUF copies are
        # DVE; the f-vector algebra stays on Pool.
        pend = {}

        def stage_a(gi):
            """DMA in + bn_stats + massage + combine matmul."""
            L, t0 = GROUPS[gi], t0s[gi]
            cols = slice(t0, t0 + L)

            # DVE: one bn_stats per 512-elem half-block
            for t in range(t0, t0 + L):
                nc.vector.bn_stats(out=bnout4[:, t, 0, :], in_=xg3[gi][:, t - t0, 0, :])
                nc.vector.bn_stats(out=bnout4[:, t, 1, :], in_=xg3[gi][:, t - t0, 1, :])
            # DVE massage: per-partition-block sums from the 4 records
            means = bnout[:, cols, :, 1]
            m2s = bnout[:, cols, :, 2]
            nc.vector.tensor_reduce(
                out=stats2[:, 0, cols], in_=means, axis=Ax.X, op=Alu.add
            )
            nc.vector.tensor_tensor(
                out=mean2[:, 0:L, :], in0=means, in1=means, op=Alu.mult
            )
            nc.vector.tensor_reduce(
                out=sm2[:, 0:L], in_=mean2[:, 0:L, :], axis=Ax.X, op=Alu.add
            )
            nc.vector.tensor_reduce(
                out=sM2[:, 0:L], in_=m2s, axis=Ax.X, op=Alu.add
            )
            nc.vector.scalar_tensor_tensor(
                out=stats2[:, 1, cols], in0=sM2[:, 0:L], scalar=1.0 / 256.0,
                in1=sm2[:, 0:L], op0=Alu.mult, op1=Alu.add,
            )
            # PE combine: [CS, 2, B] <- mask^T @ stats2 (ct layout, /4096)
            ps_mumsq = psum.tile([CS, 2, B], DT, tag=f"ps_mumsq{gi % 2}")
            nc.tensor.matmul(
                out=ps_mumsq, lhsT=sb_mask, rhs=stats2,
                start=True, stop=True,
            )
            pend[gi] = {"ps_mumsq": ps_mumsq}

        def stage_b(gi):
            """EMA scans through sqrt (smu, f, svar), all [CS, B] ct tiles."""
            st = pend[gi]
            smu_sb = smu_sbs[gi % 2]
            svar_sb = svar_sbs[gi % 2]
            sc_ct = sc_cts[gi % 2]
            st["smu"], st["sc"] = smu_sb, sc_ct

            # ACT drains PSUM: raw mu/msq plus a pre-scaled (1-a)*mu copy
            nc.scalar.activation(out=mumsq_ct, in_=st["ps_mumsq"], func=Act.Copy)
            nc.scalar.activation(
                out=muls_ct, in_=st["ps_mumsq"][:, 0, :], func=Act.Copy,
                scale=1.0 - AFWD,
            )
            mu_v = mumsq_ct[:, 0, :]
            msq_v = mumsq_ct[:, 1, :]

            # DVE scan: smu_sb[:, 1+t] = a*smu_sb[:, t] + (1-a)*mu_t
            nc.vector.tensor_tensor_scan(
                out=smu_sb[:, 1 : 1 + B], data0=sb_afwd, data1=muls_ct,
                initial=smu_sb[:, 0:1], op0=Alu.mult, op1=Alu.add,
            )

            # Pool: f = (msq - mu^2) + a*(mu - smu_prev)^2, scaled by (1-a)
            m2g = gpool.tile([CS, B], DT, tag="m2g")
            nc.gpsimd.tensor_tensor(out=m2g, in0=mu_v, in1=mu_v, op=Alu.mult)
            var_g = gpool.tile([CS, B], DT, tag="var_g")
            nc.gpsimd.tensor_tensor(out=var_g, in0=msq_v, in1=m2g, op=Alu.subtract)
            d_g = gpool.tile([CS, B], DT, tag="d_g")
            nc.gpsimd.tensor_tensor(
                out=d_g, in0=mu_v, in1=smu_sb[:, 0:B], op=Alu.subtract
            )
            ds_g = gpool.tile([CS, B], DT, tag="ds_g")
            nc.gpsimd.tensor_tensor(out=ds_g, in0=d_g, in1=sb_sqrta, op=Alu.mult)
            d2_g = gpool.tile([CS, B], DT, tag="d2_g")
            nc.gpsimd.tensor_tensor(out=d2_g, in0=ds_g, in1=ds_g, op=Alu.mult)
            f_g = gpool.tile([CS, B], DT, tag="f_g")
            nc.gpsimd.tensor_tensor(out=f_g, in0=d2_g, in1=var_g, op=Alu.add)
            fs_g = gpool.tile([CS, B], DT, tag="fs_g")
            nc.gpsimd.tensor_tensor(out=fs_g, in0=f_g, in1=sb_oma, op=Alu.mult)

            # DVE scan: svar_sb[:, 1+t] = a*svar_sb[:, t] + (1-a)*f_t
            nc.vector.tensor_tensor_scan(
                out=svar_sb[:, 1 : 1 + B], data0=sb_afwd, data1=fs_g,
                initial=svar_sb[:, 0:1], op0=Alu.mult, op1=Alu.add,
            )
            # ACT: sc = sqrt(svar_prev + eps)
            nc.scalar.activation(
                out=sc_ct, in_=svar_sb[:, 0:B], func=Act.Sqrt, bias=sb_eps
            )

        def stage_c(gi):
            """recip + nbias + broadcast + normalize + DMA out."""
            L, t0 = GROUPS[gi], t0s[gi]
            cols = slice(t0, t0 + L)
            st = pend.pop(gi)
            smu_sb, sc_ct = st["smu"], st["sc"]

            nc.vector.reciprocal(out=rs_ct, in_=sc_ct)
            # positive smu*rs here; the negation is folded into bmaskn below
            nc.gpsimd.tensor_tensor(
                out=nb_ct, in0=smu_sb[:, 0:B], in1=rs_ct, op=Alu.mult
            )
            ps_rb = psum.tile([P, 2, LMAX], DT, tag="ps_rb")
            nc.tensor.matmul(
                out=ps_rb[:, 0, 0:L], lhsT=sb_bmask, rhs=rs_ct[:, cols],
                start=True, stop=True,
            )
            nc.tensor.matmul(
                out=ps_rb[:, 1, 0:L], lhsT=sb_bmaskn, rhs=nb_ct[:, cols],
                start=True, stop=True,
            )
            nc.scalar.activation(
                out=rb[:, :, cols], in_=ps_rb[:, :, 0:L], func=Act.Copy
            )

            # normalize: early groups on ACT (overlaps the DVE bn_stats
            # stream), late groups on DVE (free after its stream ends, and
            # ~2x faster per pass than ACT)
            for t in range(t0, t0 + L):
                if gi < 4:
                    nc.scalar.activation(
                        out=xg[gi][:, t - t0, :], in_=xg[gi][:, t - t0, :],
                        func=Act.Identity,
                        bias=rb[:, 1, t : t + 1], scale=rb[:, 0, t : t + 1],
                    )
                else:
                    nc.vector.tensor_scalar(
                        out=xg[gi][:, t - t0, :], in0=xg[gi][:, t - t0, :],
                        scalar1=rb[:, 0, t : t + 1],
                        scalar2=rb[:, 1, t : t + 1],
                        op0=Alu.mult, op1=Alu.add,
                    )
            # output DMA triggered from the SP queue: a trigger's wait (on
            # this group's normalizes) must not block the Pool queue, which
            # carries the NEXT group's f-vector chain
            nc.sync.dma_start(out=out_h[:, cols, :], in_=xg[gi])

        for gi in range(NG + 1):
            if gi < NG:
                stage_a(gi)
            if gi >= 1:
                stage_b(gi - 1)
                stage_c(gi - 1)

    if not skip_compile:
        nc.compile()
    return nc


def _consts():
    mask = np.zeros((P, CS), np.float32)
    mask[np.arange(P), np.arange(P) % CS] = 1.0 / 16.0
    bmask = np.zeros((CS, P), np.float32)
    bmask[np.arange(P) % CS, np.arange(P)] = 1.0
    return {"mask": mask, "bmask": bmask, "bmaskn": -bmask}


def _in_map(x_shard, mu0_shard, var0_shard):
    """Build one core's input dict from its [P, B, F] shard + init vectors."""
    inits = np.stack([mu0_shard, var0_shard], axis=1).astype(np.float32)
    return {"x": x_shard, "inits": inits, **_consts()}


def kernel(**inputs):
    global LAST_EXEC_NS, LAST_RESULTS
    x = np.asarray(inputs["x"], dtype=np.float32)
    mu0 = np.asarray(inputs["mu0"], dtype=np.float32)
    var0 = np.asarray(inputs["var0"], dtype=np.float32)
    assert x.shape == (B, H, W_SP, C)

    from concourse.bass_utils import run_bass_kernel_spmd

    if "nc" not in _COMPILED:
        _COMPILED["nc"] = _build_bass()
    nc = _COMPILED["nc"]

    # [B, Q, F, C] view of x; per-core shard is [Q, CS, B, F] -> [P, B, F].
    # One global fp32->fp16 cast, then cheap fp16 transposed copies per core.
    xr = x.reshape(B, Q, F, C).astype(np.float16)
    in_maps = []
    for core in range(NCORES):
        c0 = core * CS
        xs = np.ascontiguousarray(
            xr[:, :, :, c0 : c0 + CS].transpose(1, 3, 0, 2)
        ).reshape(P, B, F)
        in_maps.append(
            _in_map(xs, mu0[c0 : c0 + CS], var0[c0 : c0 + CS])
        )

    trace = bool(int(os.environ.get("NORM_KERNEL_TRACE", "0")))
    if trace:
        _ensure_ntff_hook()
    res = run_bass_kernel_spmd(nc, in_maps, list(range(NCORES)), trace=trace)
    LAST_EXEC_NS = res.exec_time_ns
    LAST_RESULTS = res

    out = np.empty((B, Q, F, C), np.float32)
    for core in range(NCORES):
        c0 = core * CS
        o = res.results[core]["out"].reshape(Q, CS, B, F)
        out[:, :, :, c0 : c0 + CS] = o.transpose(2, 0, 3, 1)
    return out.reshape(B, H, W_SP, C)


# revision 48
# speedup vs baseline: 1.0692x; 1.0692x over previous
"""Online Normalization (forward) on 8 Trainium2 NeuronCores.

Reference semantics (per batch sample t, stats per channel over H*W):
    out_t = (x_t - s_mu_{t-1}) / sqrt(s_var_{t-1} + eps)
    mu_t  = mean(x_t);  var_t = mean(x_t^2) - mu_t^2
    s_mu_t  = a*s_mu_{t-1}  + (1-a)*mu_t
    s_var_t = a*s_var_{t-1} + (1-a)*var_t + a*(1-a)*(mu_t - s_mu_{t-1})^2

The EMA recurrence is linear, so per-sample batch stats feed small
lower-triangular matmuls on the tensor engine:
    s_mu_{t-1}  = a^t mu0  + sum_i W[i,t] mu_i,   W[i,t] = (1-a) a^{t-1-i}, i<t
    s_var_{t-1} = a^t var0 + sum_i W[i,t] f_i,    f_i = var_i + a*d_i^2,
                                                  d_i = mu_i - s_mu_{i-1}
The scan runs incrementally over tapered groups of samples so normalized
output streams out while later samples stream in.

Engine plan (v3): x lives in SBUF/HBM as fp16 (halves DMA traffic; the
correctness gate is 2e-2, fp16 quantization is ~4e-4).
  - DVE streams BN_STATS (mean+M2 per 512-elem block in one pass -- this
    replaces separate sum and square passes) plus a few small per-group
    reductions; nothing else sits in its queue except one tiny reciprocal
    per group, issued one group late so it never stalls the stream.
  - ACT streams all 32 normalizes (Identity w/ per-partition scale+bias)
    plus one small Sqrt per group.
  - Pool (gpsimd) runs the small PSUM<->SBUF copies and f-vector algebra
    of the stats chain, and triggers the output DMAs (SWDGE).
  - PE does the stats matmuls in [t, c] layout: operand-swapped combine
    (no transposes needed until the final [c, t] flip), with the mu0/var0
    init and eps folded in as extra contraction rows.

Sharding: channels C=256 split across 8 cores (32 each). Per core the
8 MiB fp16 shard is [128 partitions, 32 t, 1024 f], partition p = q*32+c
(q = one of 4 spatial blocks, c = channel).
"""

import os
import sys

import numpy as np

sys.path.insert(0, "/opt/trn_rl_repo")

B = 32          # batch (sequential scan axis)
H = 64
W_SP = 64
C = 256
NCORES = 8
CS = C // NCORES    # 32 channels per core
Q = 4               # spatial blocks per sample
F = (H * W_SP) // Q  # 1024 elements per block
P = 128             # partitions (Q*CS)
AFWD = 0.999
EPS = 1e-5
# tapered scan groups (= DMA chunk sizes, in batch samples): tiny head for
# fast pipeline fill, small tail so the final chain+normalize drains fast
GROUPS = [1, 3, 8, 8, 8, 4]
assert sum(GROUPS) == B
# normalize engine split: early groups ride ACT while DVE streams bn_stats;
# the last two groups are split with DVE (free after its stream, 2x faster)
NORM_DVE = {4: 5, 5: 3}   # group -> how many of its samples normalize on DVE

LAST_EXEC_NS = None
LAST_RESULTS = None
_COMPILED = {}


def _ensure_ntff_hook():
    """The axon boot degrades silently when ``antenv.axon_hooks`` is missing;
    provide the module + the ctypes-based NRT-profile hook ourselves so
    ``run_bass_kernel_spmd(trace=True)`` can capture NTFF profiles."""
    try:
        from antenv.axon_hooks import get_axon_ntff_profile_hook  # noqa: F401

        return
    except ImportError:
        pass

    import contextlib
    import ctypes
    import types

    so_path = "/opt/axon/libaxon_pjrt.so"
    state = {"hook": None}

    mod = types.ModuleType("antenv.axon_hooks")

    def set_axon_ntff_profile_hook(h):
        state["hook"] = h

    def get_axon_ntff_profile_hook():
        return state["hook"]

    mod.set_axon_ntff_profile_hook = set_axon_ntff_profile_hook
    mod.get_axon_ntff_profile_hook = get_axon_ntff_profile_hook
    import antenv

    antenv.axon_hooks = mod
    sys.modules["antenv.axon_hooks"] = mod

    if not os.path.exists(so_path):
        return
    lib = ctypes.CDLL(so_path)
    if not hasattr(lib, "axon_start_nrt_profile"):
        return
    lib.axon_start_nrt_profile.argtypes = [
        ctypes.POINTER(ctypes.c_int64),
        ctypes.c_size_t,
    ]
    lib.axon_start_nrt_profile.restype = ctypes.c_int64
    lib.axon_stop_nrt_profile.argtypes = [ctypes.c_char_p]
    lib.axon_stop_nrt_profile.restype = ctypes.c_int64

    @contextlib.contextmanager
    def _hook(output_dir, device_ids):
        import jax

        jax.devices()
        if device_ids:
            ids = (ctypes.c_int64 * len(device_ids))(*device_ids)
            rc = lib.axon_start_nrt_profile(ids, len(device_ids))
        else:
            rc = lib.axon_start_nrt_profile(None, 0)
        if rc != 0:
            raise RuntimeError(f"axon_start_nrt_profile rc={rc}")
        try:
            yield
        finally:
            n = lib.axon_stop_nrt_profile(str(output_dir).encode())
            print(f"profile: {n} file(s) written to {output_dir}", file=sys.stderr)

    state["hook"] = _hook


def _patch_fishpath():
    """The _compat FishPath shim lacks pathlib conveniences the manifest
    capture/replay helpers use."""
    import pathlib

    from concourse import _compat

    def _open(self, mode="r"):
        p = pathlib.Path(str(self))
        if "w" in mode:
            p.parent.mkdir(parents=True, exist_ok=True)
        return open(str(p), mode)

    _compat.FishPath.open = _open
    _compat.FishPath.mkdir = lambda self, **kw: pathlib.Path(str(self)).mkdir(**kw)
    _compat.FishPath.__fspath__ = lambda self: str(self)
    if not hasattr(_compat.FishPath, "parent"):
        _compat.FishPath.parent = property(
            lambda self: _compat.FishPath(pathlib.Path(str(self)).parent)
        )
    if not hasattr(_compat.FishPath, "stem"):
        _compat.FishPath.stem = property(
            lambda self: pathlib.Path(str(self)).stem
        )


def _manifest_capture_main():
    """Subprocess entry: build (schedule-only) under
    TILE_CAPTURE_MANIFEST_PATH so the schedule manifest lands on disk."""
    _patch_fishpath()
    try:
        _build_bass_raw(skip_compile=True)
    except Exception as e:  # manifest is written before trailing debug steps
        print(f"capture pass ended with: {e}", file=sys.stderr)


def _edit_manifest(path):
    """Rewrite the captured schedule order to pure issue order (sort by
    instruction number). The issue order is hand-pipelined so that every
    small cross-engine chain op sits right after the bn_stats group that
    feeds it; the CoreSim list scheduler instead floats those ops ~2 groups
    late, which serializes the whole back half of the kernel."""
    import json
    import re

    with open(path) as f:
        d = json.load(f)
    for block, order in d["order"].items():
        order.sort(key=lambda e: int(re.match(r"I-(\d+)", e["name"]).group(1)))
    with open(path, "w") as f:
        json.dump(d, f)


def _build_bass():
    import glob
    import subprocess
    import tempfile

    mdir = tempfile.mkdtemp(prefix="norm_manifest_")
    here = os.path.dirname(os.path.abspath(__file__))
    env = {**os.environ, "TILE_CAPTURE_MANIFEST_PATH": mdir}
    env.pop("TILE_SCHEDULER", None)
    env.pop("TILE_LOAD_MANIFEST_PATH", None)
    try:
        subprocess.run(
            [
                sys.executable,
                "-c",
                f"import sys; sys.path.insert(0, {here!r}); "
                "import kernel; kernel._manifest_capture_main()",
            ],
            env=env,
            timeout=600,
            check=False,
        )
        manifests = glob.glob(os.path.join(mdir, "*.json"))
        assert len(manifests) == 1, f"expected 1 manifest, got {manifests}"
        _edit_manifest(manifests[0])
        _patch_fishpath()
        os.environ["TILE_SCHEDULER"] = "manifest"
        os.environ["TILE_LOAD_MANIFEST_PATH"] = mdir
        try:
            return _build_bass_raw()
        finally:
            os.environ.pop("TILE_SCHEDULER", None)
            os.environ.pop("TILE_LOAD_MANIFEST_PATH", None)
    except Exception as e:
        print(f"manifest schedule failed ({e}); default scheduler", file=sys.stderr)
        os.environ.pop("TILE_SCHEDULER", None)
        os.environ.pop("TILE_LOAD_MANIFEST_PATH", None)
        return _build_bass_raw()


def _build_bass_raw(skip_compile=False):
    from contextlib import ExitStack

    import concourse.bacc as bacc
    import concourse.tile as tile
    from concourse import mybir

    DT = mybir.dt.float32
    DT16 = mybir.dt.float16
    Alu = mybir.AluOpType
    Act = mybir.ActivationFunctionType
    Ax = mybir.AxisListType

    nc = bacc.Bacc(
        "TRN2", target_bir_lowering=False, debug=False, num_devices=NCORES
    )
    x_h = nc.declare_dram_parameter("x", [P, B, F], DT16, isOutput=False)
    mask_h = nc.declare_dram_parameter("mask", [P, CS], DT, isOutput=False)
    bmask_h = nc.declare_dram_parameter("bmask", [CS, P], DT, isOutput=False)
    bmaskn_h = nc.declare_dram_parameter("bmaskn", [CS, P], DT, isOutput=False)
    inits_h = nc.declare_dram_parameter("inits", [CS, 2], DT, isOutput=False)
    out_h = nc.declare_dram_parameter("out", [P, B, F], DT16, isOutput=True)

    NG = len(GROUPS)
    LMAX = max(GROUPS)

    with tile.TileContext(nc) as tc, ExitStack() as ctx:
        consts = ctx.enter_context(tc.tile_pool(name="consts", bufs=1))
        xpool = ctx.enter_context(tc.tile_pool(name="xp", bufs=1))
        small = ctx.enter_context(tc.tile_pool(name="small", bufs=1))
        gpool = ctx.enter_context(tc.tile_pool(name="gp", bufs=2))
        psum = ctx.enter_context(tc.tile_pool(name="ps", bufs=1, space="PSUM"))

        # one tile per group: per-group input DMAs, bn_stats reads, in-place
        # normalizes, and output DMAs then carry NO false dependencies on
        # other groups' data. Trigger the first two groups' input DMAs ahead
        # of the const loads so the bn_stats stream starts as early as
        # possible; the consts are only needed ~15us in.
        xg = [
            xpool.tile([P, L, F], DT16, tag=f"xg{i}", name=f"xg{i}")
            for i, L in enumerate(GROUPS)
        ]
        xg3 = [t.rearrange("p b (two f) -> p b two f", two=2) for t in xg]
        t0s = []
        t0 = 0
        for L in GROUPS:
            t0s.append(t0)
            t0 += L
        for gi in (0, 1):
            nc.sync.dma_start(
                out=xg[gi], in_=x_h[:, t0s[gi] : t0s[gi] + GROUPS[gi], :]
            )

        sb_mask = consts.tile([P, CS], DT)       # mask[p, c] = [p%CS==c]/16
        nc.sync.dma_start(out=sb_mask, in_=mask_h[:, :])
        sb_bmask = consts.tile([CS, P], DT)      # bmask[c, p] = [p%CS==c]
        nc.sync.dma_start(out=sb_bmask, in_=bmask_h[:, :])
        sb_bmaskn = consts.tile([CS, P], DT)     # -bmask (negates nbias)
        nc.sync.dma_start(out=sb_bmaskn, in_=bmaskn_h[:, :])
        sb_sqrta = consts.tile([CS, B], DT)      # sqrt(AFWD): f = (sqrt(a)d)^2+var
        nc.vector.memset(sb_sqrta, float(AFWD ** 0.5))
        sb_afwd = consts.tile([CS, B], DT)       # scan multiplier a
        nc.vector.memset(sb_afwd, AFWD)
        sb_oma = consts.tile([CS, B], DT)        # 1-a (scales f for the var scan)
        nc.vector.memset(sb_oma, 1.0 - AFWD)
        sb_eps = consts.tile([CS, 1], DT)
        nc.vector.memset(sb_eps, EPS)

        for gi in range(2, len(GROUPS)):
            nc.sync.dma_start(
                out=xg[gi], in_=x_h[:, t0s[gi] : t0s[gi] + GROUPS[gi], :]
            )

        # bn_stats records: per sample 2 blocks x (even, odd) halves
        # = 4 records of (count, mean, M2)
        bnout = small.tile([P, B, 4, 3], DT)
        bnout4 = bnout.rearrange("p b (k two) three -> p b k (two three)", two=2)
        mean2 = small.tile([P, LMAX, 4], DT)
        sm2 = small.tile([P, LMAX], DT)
        sM2 = small.tile([P, LMAX], DT)
        # stats2[:, 0, t] = sum_x/256 per partition-block; [:, 1, t] = sum_x2/256
        stats2 = small.tile([P, 2, B], DT)
        nc.vector.memset(stats2, 0.0)

        # [c, t] layout state. The EMA recurrences run as tensor_tensor_scan
        # along the free (t) axis with fp32 internal state -- exactly the
        # reference recurrence, no W matrices and no transposes. Column 0 of
        # each scan tile holds the initial state (mu0 / var0), so columns
        # 0..B-1 of the tile ARE the "previous" states the outputs need.
        # smu/sc are double-buffered across groups (written in stage_b(g+1)
        # while stage_c(g) still reads them).
        mumsq_ct = small.tile([CS, 2, B], DT)    # raw mu / msq, ct layout
        muls_ct = small.tile([CS, B], DT)        # (1-a) * mu
        smu_sbs, svar_sbs, sc_cts = [], [], []
        for k in range(2):
            t_smu = small.tile([CS, 1 + B], DT, name=f"smu_sb{k}")
            nc.sync.dma_start(out=t_smu[:, 0:1], in_=inits_h[:, 0:1])
            smu_sbs.append(t_smu)
            t_svar = small.tile([CS, 1 + B], DT, name=f"svar_sb{k}")
            nc.sync.dma_start(out=t_svar[:, 0:1], in_=inits_h[:, 1:2])
            svar_sbs.append(t_svar)
            t_sc = small.tile([CS, B], DT, name=f"sc_ct{k}")
            sc_cts.append(t_sc)
        rs_ct = small.tile([CS, B], DT)
        nb_ct = small.tile([CS, B], DT)
        rb = small.tile([P, 2, B], DT)          # [:,0,t]=rscale, [:,1,t]=nbias

        # warm the sqrt_and_others activation table before the streaming
        # phase so no ACT_TABLE_LOAD lands mid-kernel
        warm = small.tile([1, 1], DT)
        nc.vector.memset(warm, 1.0)
        nc.scalar.activation(out=warm, in_=warm, func=Act.Sqrt)

        t0s = []
        t0 = 0
        for L in GROUPS:
            t0s.append(t0)
            t0 += L

        # Fine-grained software pipeline, replayed verbatim via the schedule
        # manifest (the CoreSim list scheduler would float the small chain
        # ops ~2 groups late, serializing the back half). Per slot s the
        # chain of group s-1 is woven BETWEEN the bn_stats of group s with
        # enough spacing that every op's cross-engine producers are done by
        # the time its engine reaches it; normalizes run two slots behind.
        pend = {}

        def emit_bn(gi, lo, hi):
            L, t0 = GROUPS[gi], t0s[gi]
            for j in range(min(lo, 2 * L), min(hi, 2 * L)):
                t, k = t0 + j // 2, j % 2
                nc.vector.bn_stats(
                    out=bnout4[:, t, k, :], in_=xg3[gi][:, t - t0, k, :]
                )

        def emit_massage_mm1(gi):
            L, t0 = GROUPS[gi], t0s[gi]
            cols = slice(t0, t0 + L)
            means = bnout[:, cols, :, 1]
            m2s = bnout[:, cols, :, 2]
            nc.vector.tensor_reduce(
                out=stats2[:, 0, cols], in_=means, axis=Ax.X, op=Alu.add
            )
            nc.vector.tensor_tensor(
                out=mean2[:, 0:L, :], in0=means, in1=means, op=Alu.mult
            )
            nc.vector.tensor_reduce(
                out=sm2[:, 0:L], in_=mean2[:, 0:L, :], axis=Ax.X, op=Alu.add
            )
            nc.vector.tensor_reduce(
                out=sM2[:, 0:L], in_=m2s, axis=Ax.X, op=Alu.add
            )
            nc.vector.scalar_tensor_tensor(
                out=stats2[:, 1, cols], in0=sM2[:, 0:L], scalar=1.0 / 256.0,
                in1=sm2[:, 0:L], op0=Alu.mult, op1=Alu.add,
            )
            ps_mumsq = psum.tile([CS, 2, B], DT, tag=f"ps_mumsq{gi % 2}")
            nc.tensor.matmul(
                out=ps_mumsq, lhsT=sb_mask, rhs=stats2, start=True, stop=True
            )
            pend[gi] = {
                "ps_mumsq": ps_mumsq,
                "smu": smu_sbs[gi % 2],
                "svar": svar_sbs[gi % 2],
                "sc": sc_cts[gi % 2],
            }

        def emit_cp(gi):
            st = pend[gi]
            nc.scalar.activation(out=mumsq_ct, in_=st["ps_mumsq"], func=Act.Copy)
            nc.scalar.activation(
                out=muls_ct, in_=st["ps_mumsq"][:, 0, :], func=Act.Copy,
                scale=1.0 - AFWD,
            )

        def emit_s1_f(gi):
            st = pend[gi]
            smu_sb = st["smu"]
            nc.vector.tensor_tensor_scan(
                out=smu_sb[:, 1 : 1 + B], data0=sb_afwd, data1=muls_ct,
                initial=smu_sb[:, 0:1], op0=Alu.mult, op1=Alu.add,
            )
            mu_v = mumsq_ct[:, 0, :]
            msq_v = mumsq_ct[:, 1, :]
            m2g = gpool.tile([CS, B], DT, tag="m2g")
            nc.gpsimd.tensor_tensor(out=m2g, in0=mu_v, in1=mu_v, op=Alu.mult)
            var_g = gpool.tile([CS, B], DT, tag="var_g")
            nc.gpsimd.tensor_tensor(out=var_g, in0=msq_v, in1=m2g, op=Alu.subtract)
            d_g = gpool.tile([CS, B], DT, tag="d_g")
            nc.gpsimd.tensor_tensor(
                out=d_g, in0=mu_v, in1=smu_sb[:, 0:B], op=Alu.subtract
            )
            ds_g = gpool.tile([CS, B], DT, tag="ds_g")
            nc.gpsimd.tensor_tensor(out=ds_g, in0=d_g, in1=sb_sqrta, op=Alu.mult)
            d2_g = gpool.tile([CS, B], DT, tag="d2_g")
            nc.gpsimd.tensor_tensor(out=d2_g, in0=ds_g, in1=ds_g, op=Alu.mult)
            f_g = gpool.tile([CS, B], DT, tag="f_g")
            nc.gpsimd.tensor_tensor(out=f_g, in0=d2_g, in1=var_g, op=Alu.add)
            fs_g = gpool.tile([CS, B], DT, tag="fs_g")
            nc.gpsimd.tensor_tensor(out=fs_g, in0=f_g, in1=sb_oma, op=Alu.mult)
            st["fs"] = fs_g

        def emit_s2_sqrt(gi):
            st = pend[gi]
            svar_sb = st["svar"]
            nc.vector.tensor_tensor_scan(
                out=svar_sb[:, 1 : 1 + B], data0=sb_afwd, data1=st["fs"],
                initial=svar_sb[:, 0:1], op0=Alu.mult, op1=Alu.add,
            )
            nc.scalar.activation(
                out=st["sc"], in_=svar_sb[:, 0:B], func=Act.Sqrt, bias=sb_eps
            )

        def emit_rc_rb(gi):
            L, t0 = GROUPS[gi], t0s[gi]
            cols = slice(t0, t0 + L)
            st = pend[gi]
            nc.vector.reciprocal(out=rs_ct, in_=st["sc"])
            # positive smu*rs; the negation is folded into bmaskn
            nc.gpsimd.tensor_tensor(
                out=nb_ct, in0=st["smu"][:, 0:B], in1=rs_ct, op=Alu.mult
            )
            ps_rb = psum.tile([P, 2, LMAX], DT, tag="ps_rb")
            nc.tensor.matmul(
                out=ps_rb[:, 0, 0:L], lhsT=sb_bmask, rhs=rs_ct[:, cols],
                start=True, stop=True,
            )
            nc.tensor.matmul(
                out=ps_rb[:, 1, 0:L], lhsT=sb_bmaskn, rhs=nb_ct[:, cols],
                start=True, stop=True,
            )
            nc.scalar.activation(
                out=rb[:, :, cols], in_=ps_rb[:, :, 0:L], func=Act.Copy
            )

        def emit_norms_out(gi):
            L, t0 = GROUPS[gi], t0s[gi]
            cols = slice(t0, t0 + L)
            pend.pop(gi, None)
            n_dve = NORM_DVE.get(gi, 0)
            for t in range(t0, t0 + L - n_dve):
                nc.scalar.activation(
                    out=xg[gi][:, t - t0, :], in_=xg[gi][:, t - t0, :],
                    func=Act.Identity,
                    bias=rb[:, 1, t : t + 1], scale=rb[:, 0, t : t + 1],
                )
            for t in range(t0 + L - n_dve, t0 + L):
                nc.vector.tensor_scalar(
                    out=xg[gi][:, t - t0, :], in0=xg[gi][:, t - t0, :],
                    scalar1=rb[:, 0, t : t + 1], scalar2=rb[:, 1, t : t + 1],
                    op0=Alu.mult, op1=Alu.add,
                )
            nc.sync.dma_start(out=out_h[:, cols, :], in_=xg[gi])

        for s in range(NG + 2):
            a = s if s < NG else None         # group streaming in
            b = s - 1 if 1 <= s <= NG else None   # group running its chain
            c = s - 2 if s >= 2 else None     # group normalizing + storing
            if a is not None:
                emit_bn(a, 0, 2)
            if b is not None:
                emit_cp(b)
            if a is not None:
                emit_bn(a, 2, 4)
            if b is not None:
                emit_s1_f(b)
            if a is not None:
                emit_bn(a, 4, 8)
            if b is not None:
                emit_s2_sqrt(b)
            if a is not None:
                emit_bn(a, 8, 10)
            if b is not None:
                emit_rc_rb(b)
            if a is not None:
                emit_bn(a, 10, 2 * GROUPS[a])
            if c is not None:
                emit_norms_out(c)
            if a is not None:
                emit_massage_mm1(a)

    if not skip_compile:
        nc.compile()
    return nc


def _consts():
    mask = np.zeros((P, CS), np.float32)
    mask[np.arange(P), np.arange(P) % CS] = 1.0 / 16.0
    bmask = np.zeros((CS, P), np.float32)
    bmask[np.arange(P) % CS, np.arange(P)] = 1.0
    return {"mask": mask, "bmask": bmask, "bmaskn": -bmask}


def _in_map(x_shard, mu0_shard, var0_shard):
    """Build one core's input dict from its [P, B, F] shard + init vectors."""
    inits = np.stack([mu0_shard, var0_shard], axis=1).astype(np.float32)
    return {"x": x_shard, "inits": inits, **_consts()}


def kernel(**inputs):
    global LAST_EXEC_NS, LAST_RESULTS
    x = np.asarray(inputs["x"], dtype=np.float32)
    mu0 = np.asarray(inputs["mu0"], dtype=np.float32)
    var0 = np.asarray(inputs["var0"], dtype=np.float32)
    assert x.shape == (B, H, W_SP, C)

    from concourse.bass_utils import run_bass_kernel_spmd

    if "nc" not in _COMPILED:
        _COMPILED["nc"] = _build_bass()
    nc = _COMPILED["nc"]

    # [B, Q, F, C] view of x; per-core shard is [Q, CS, B, F] -> [P, B, F].
    # One global fp32->fp16 cast, then cheap fp16 transposed copies per core.
    xr = x.reshape(B, Q, F, C).astype(np.float16)
    in_maps = []
    for core in range(NCORES):
        c0 = core * CS
        xs = np.ascontiguousarray(
            xr[:, :, :, c0 : c0 + CS].transpose(1, 3, 0, 2)
        ).reshape(P, B, F)
        in_maps.append(
            _in_map(xs, mu0[c0 : c0 + CS], var0[c0 : c0 + CS])
        )

    trace = bool(int(os.environ.get("NORM_KERNEL_TRACE", "0")))
    if trace:
        _ensure_ntff_hook()
    res = run_bass_kernel_spmd(nc, in_maps, list(range(NCORES)), trace=trace)
    LAST_EXEC_NS = res.exec_time_ns
    LAST_RESULTS = res

    out = np.empty((B, Q, F, C), np.float32)
    for core in range(NCORES):
        c0 = core * CS
        o = res.results[core]["out"].reshape(Q, CS, B, F)
        out[:, :, :, c0 : c0 + CS] = o.transpose(2, 0, 3, 1)
    return out.reshape(B, H, W_SP, C)


# revision 49
# speedup vs baseline: 1.3568x; 1.2690x over previous
"""Online Normalization (forward) on 8 Trainium2 NeuronCores.

Reference semantics (per batch sample t, stats per channel over H*W):
    out_t = (x_t - s_mu_{t-1}) / sqrt(s_var_{t-1} + eps)
    mu_t  = mean(x_t);  var_t = mean(x_t^2) - mu_t^2
    s_mu_t  = a*s_mu_{t-1}  + (1-a)*mu_t
    s_var_t = a*s_var_{t-1} + (1-a)*var_t + a*(1-a)*(mu_t - s_mu_{t-1})^2

The EMA recurrence is linear, so per-sample batch stats feed small
lower-triangular matmuls on the tensor engine:
    s_mu_{t-1}  = a^t mu0  + sum_i W[i,t] mu_i,   W[i,t] = (1-a) a^{t-1-i}, i<t
    s_var_{t-1} = a^t var0 + sum_i W[i,t] f_i,    f_i = var_i + a*d_i^2,
                                                  d_i = mu_i - s_mu_{i-1}
The scan runs incrementally over tapered groups of samples so normalized
output streams out while later samples stream in.

Engine plan (v3): x lives in SBUF/HBM as fp16 (halves DMA traffic; the
correctness gate is 2e-2, fp16 quantization is ~4e-4).
  - DVE streams BN_STATS (mean+M2 per 512-elem block in one pass -- this
    replaces separate sum and square passes) plus a few small per-group
    reductions; nothing else sits in its queue except one tiny reciprocal
    per group, issued one group late so it never stalls the stream.
  - ACT streams all 32 normalizes (Identity w/ per-partition scale+bias)
    plus one small Sqrt per group.
  - Pool (gpsimd) runs the small PSUM<->SBUF copies and f-vector algebra
    of the stats chain, and triggers the output DMAs (SWDGE).
  - PE does the stats matmuls in [t, c] layout: operand-swapped combine
    (no transposes needed until the final [c, t] flip), with the mu0/var0
    init and eps folded in as extra contraction rows.

Sharding: channels C=256 split across 8 cores (32 each). Per core the
8 MiB fp16 shard is [128 partitions, 32 t, 1024 f], partition p = q*32+c
(q = one of 4 spatial blocks, c = channel).
"""

import os
import sys

import numpy as np

sys.path.insert(0, "/opt/trn_rl_repo")

B = 32          # batch (sequential scan axis)
H = 64
W_SP = 64
C = 256
NCORES = 8
CS = C // NCORES    # 32 channels per core
Q = 4               # spatial blocks per sample
F = (H * W_SP) // Q  # 1024 elements per block
P = 128             # partitions (Q*CS)
AFWD = 0.999
EPS = 1e-5
# tapered scan groups (= DMA chunk sizes, in batch samples): tiny head for
# fast pipeline fill, small tail so the final chain+normalize drains fast
GROUPS = [1, 3, 8, 8, 8, 4]
assert sum(GROUPS) == B
# normalize engine split: early groups ride ACT while DVE streams bn_stats;
# the last two groups are split with DVE (free after its stream, 2x faster)
NORM_DVE = {4: 5, 5: 3}   # group -> how many of its samples normalize on DVE

LAST_EXEC_NS = None
LAST_RESULTS = None
_COMPILED = {}


def _ensure_ntff_hook():
    """The axon boot degrades silently when ``antenv.axon_hooks`` is missing;
    provide the module + the ctypes-based NRT-profile hook ourselves so
    ``run_bass_kernel_spmd(trace=True)`` can capture NTFF profiles."""
    try:
        from antenv.axon_hooks import get_axon_ntff_profile_hook  # noqa: F401

        return
    except ImportError:
        pass

    import contextlib
    import ctypes
    import types

    so_path = "/opt/axon/libaxon_pjrt.so"
    state = {"hook": None}

    mod = types.ModuleType("antenv.axon_hooks")

    def set_axon_ntff_profile_hook(h):
        state["hook"] = h

    def get_axon_ntff_profile_hook():
        return state["hook"]

    mod.set_axon_ntff_profile_hook = set_axon_ntff_profile_hook
    mod.get_axon_ntff_profile_hook = get_axon_ntff_profile_hook
    import antenv

    antenv.axon_hooks = mod
    sys.modules["antenv.axon_hooks"] = mod

    if not os.path.exists(so_path):
        return
    lib = ctypes.CDLL(so_path)
    if not hasattr(lib, "axon_start_nrt_profile"):
        return
    lib.axon_start_nrt_profile.argtypes = [
        ctypes.POINTER(ctypes.c_int64),
        ctypes.c_size_t,
    ]
    lib.axon_start_nrt_profile.restype = ctypes.c_int64
    lib.axon_stop_nrt_profile.argtypes = [ctypes.c_char_p]
    lib.axon_stop_nrt_profile.restype = ctypes.c_int64

    @contextlib.contextmanager
    def _hook(output_dir, device_ids):
        import jax

        jax.devices()
        if device_ids:
            ids = (ctypes.c_int64 * len(device_ids))(*device_ids)
            rc = lib.axon_start_nrt_profile(ids, len(device_ids))
        else:
            rc = lib.axon_start_nrt_profile(None, 0)
        if rc != 0:
            raise RuntimeError(f"axon_start_nrt_profile rc={rc}")
        try:
            yield
        finally:
            n = lib.axon_stop_nrt_profile(str(output_dir).encode())
            print(f"profile: {n} file(s) written to {output_dir}", file=sys.stderr)

    state["hook"] = _hook


def _patch_fishpath():
    """The _compat FishPath shim lacks pathlib conveniences the manifest
    capture/replay helpers use."""
    import pathlib

    from concourse import _compat

    def _open(self, mode="r"):
        p = pathlib.Path(str(self))
        if "w" in mode:
            p.parent.mkdir(parents=True, exist_ok=True)
        return open(str(p), mode)

    _compat.FishPath.open = _open
    _compat.FishPath.mkdir = lambda self, **kw: pathlib.Path(str(self)).mkdir(**kw)
    _compat.FishPath.__fspath__ = lambda self: str(self)
    if not hasattr(_compat.FishPath, "parent"):
        _compat.FishPath.parent = property(
            lambda self: _compat.FishPath(pathlib.Path(str(self)).parent)
        )
    if not hasattr(_compat.FishPath, "stem"):
        _compat.FishPath.stem = property(
            lambda self: pathlib.Path(str(self)).stem
        )


def _manifest_capture_main():
    """Subprocess entry: build (schedule-only) under
    TILE_CAPTURE_MANIFEST_PATH so the schedule manifest lands on disk."""
    _patch_fishpath()
    try:
        _build_bass_raw(skip_compile=True)
    except Exception as e:  # manifest is written before trailing debug steps
        print(f"capture pass ended with: {e}", file=sys.stderr)


def _edit_manifest(path):
    """Rewrite the captured schedule order to pure issue order (sort by
    instruction number). The issue order is hand-pipelined so that every
    small cross-engine chain op sits right after the bn_stats group that
    feeds it; the CoreSim list scheduler instead floats those ops ~2 groups
    late, which serializes the whole back half of the kernel."""
    import json
    import re

    with open(path) as f:
        d = json.load(f)
    for block, order in d["order"].items():
        order.sort(key=lambda e: int(re.match(r"I-(\d+)", e["name"]).group(1)))
    with open(path, "w") as f:
        json.dump(d, f)


def _build_bass():
    # The CoreSim list scheduler handles the DMA queue interleave well; a
    # hand-ordered manifest replay was tried and regressed DMA pacing.
    return _build_bass_raw()


def _build_bass_raw(skip_compile=False):
    from contextlib import ExitStack

    import concourse.bacc as bacc
    import concourse.tile as tile
    from concourse import mybir

    DT = mybir.dt.float32
    DT16 = mybir.dt.float16
    Alu = mybir.AluOpType
    Act = mybir.ActivationFunctionType
    Ax = mybir.AxisListType

    nc = bacc.Bacc(
        "TRN2", target_bir_lowering=False, debug=False, num_devices=NCORES
    )
    x_h = nc.declare_dram_parameter("x", [P, B, F], DT16, isOutput=False)
    mask_h = nc.declare_dram_parameter("mask", [P, CS], DT, isOutput=False)
    bmask_h = nc.declare_dram_parameter("bmask", [CS, P], DT, isOutput=False)
    bmaskn_h = nc.declare_dram_parameter("bmaskn", [CS, P], DT, isOutput=False)
    inits_h = nc.declare_dram_parameter("inits", [CS, 2], DT, isOutput=False)
    out_h = nc.declare_dram_parameter("out", [P, B, F], DT16, isOutput=True)

    NG = len(GROUPS)
    LMAX = max(GROUPS)

    with tile.TileContext(nc) as tc, ExitStack() as ctx:
        consts = ctx.enter_context(tc.tile_pool(name="consts", bufs=1))
        xpool = ctx.enter_context(tc.tile_pool(name="xp", bufs=1))
        small = ctx.enter_context(tc.tile_pool(name="small", bufs=1))
        gpool = ctx.enter_context(tc.tile_pool(name="gp", bufs=2))
        psum = ctx.enter_context(tc.tile_pool(name="ps", bufs=1, space="PSUM"))

        # one tile per group: per-group input DMAs, bn_stats reads, in-place
        # normalizes, and output DMAs then carry NO false dependencies on
        # other groups' data. Trigger the first two groups' input DMAs ahead
        # of the const loads so the bn_stats stream starts as early as
        # possible; the consts are only needed ~15us in.
        xg = [
            xpool.tile([P, L, F], DT16, tag=f"xg{i}", name=f"xg{i}")
            for i, L in enumerate(GROUPS)
        ]
        xg3 = [t.rearrange("p b (two f) -> p b two f", two=2) for t in xg]
        t0s = []
        t0 = 0
        for L in GROUPS:
            t0s.append(t0)
            t0 += L
        for gi in (0, 1):
            nc.sync.dma_start(
                out=xg[gi], in_=x_h[:, t0s[gi] : t0s[gi] + GROUPS[gi], :]
            )

        sb_mask = consts.tile([P, CS], DT)       # mask[p, c] = [p%CS==c]/16
        nc.sync.dma_start(out=sb_mask, in_=mask_h[:, :])
        sb_bmask = consts.tile([CS, P], DT)      # bmask[c, p] = [p%CS==c]
        nc.sync.dma_start(out=sb_bmask, in_=bmask_h[:, :])
        sb_bmaskn = consts.tile([CS, P], DT)     # -bmask (negates nbias)
        nc.sync.dma_start(out=sb_bmaskn, in_=bmaskn_h[:, :])
        sb_sqrta = consts.tile([CS, B], DT)      # sqrt(AFWD): f = (sqrt(a)d)^2+var
        nc.vector.memset(sb_sqrta, float(AFWD ** 0.5))
        sb_afwd = consts.tile([CS, B], DT)       # scan multiplier a
        nc.vector.memset(sb_afwd, AFWD)
        sb_oma = consts.tile([CS, B], DT)        # 1-a (scales f for the var scan)
        nc.vector.memset(sb_oma, 1.0 - AFWD)
        sb_eps = consts.tile([CS, 1], DT)
        nc.vector.memset(sb_eps, EPS)

        for gi in range(2, len(GROUPS)):
            nc.sync.dma_start(
                out=xg[gi], in_=x_h[:, t0s[gi] : t0s[gi] + GROUPS[gi], :]
            )

        # bn_stats records: per sample 2 blocks x (even, odd) halves
        # = 4 records of (count, mean, M2)
        bnout = small.tile([P, B, 4, 3], DT)
        bnout4 = bnout.rearrange("p b (k two) three -> p b k (two three)", two=2)
        mean2 = small.tile([P, LMAX, 4], DT)
        sm2 = small.tile([P, LMAX], DT)
        sM2 = small.tile([P, LMAX], DT)
        # stats2[:, 0, t] = sum_x/256 per partition-block; [:, 1, t] = sum_x2/256
        stats2 = small.tile([P, 2, B], DT)
        nc.vector.memset(stats2, 0.0)

        # [c, t] layout state. The EMA recurrences run as tensor_tensor_scan
        # along the free (t) axis with fp32 internal state -- exactly the
        # reference recurrence, no W matrices and no transposes. Column 0 of
        # each scan tile holds the initial state (mu0 / var0), so columns
        # 0..B-1 of the tile ARE the "previous" states the outputs need.
        # smu/sc are double-buffered across groups (written in stage_b(g+1)
        # while stage_c(g) still reads them).
        mumsq_ct = small.tile([CS, 2, B], DT)    # raw mu / msq, ct layout
        muls_ct = small.tile([CS, B], DT)        # (1-a) * mu
        smu_sbs, svar_sbs, sc_cts = [], [], []
        for k in range(2):
            t_smu = small.tile([CS, 1 + B], DT, name=f"smu_sb{k}")
            nc.sync.dma_start(out=t_smu[:, 0:1], in_=inits_h[:, 0:1])
            smu_sbs.append(t_smu)
            t_svar = small.tile([CS, 1 + B], DT, name=f"svar_sb{k}")
            nc.sync.dma_start(out=t_svar[:, 0:1], in_=inits_h[:, 1:2])
            svar_sbs.append(t_svar)
            t_sc = small.tile([CS, B], DT, name=f"sc_ct{k}")
            sc_cts.append(t_sc)
        rs_ct = small.tile([CS, B], DT)
        nb_ct = small.tile([CS, B], DT)
        rb = small.tile([P, 2, B], DT)          # [:,0,t]=rscale, [:,1,t]=nbias

        # warm the sqrt_and_others activation table before the streaming
        # phase so no ACT_TABLE_LOAD lands mid-kernel
        warm = small.tile([1, 1], DT)
        nc.vector.memset(warm, 1.0)
        nc.scalar.activation(out=warm, in_=warm, func=Act.Sqrt)

        t0s = []
        t0 = 0
        for L in GROUPS:
            t0s.append(t0)
            t0 += L

        # Fine-grained software pipeline, replayed verbatim via the schedule
        # manifest (the CoreSim list scheduler would float the small chain
        # ops ~2 groups late, serializing the back half). Per slot s the
        # chain of group s-1 is woven BETWEEN the bn_stats of group s with
        # enough spacing that every op's cross-engine producers are done by
        # the time its engine reaches it; normalizes run two slots behind.
        pend = {}

        def emit_bn(gi, lo, hi):
            L, t0 = GROUPS[gi], t0s[gi]
            for j in range(min(lo, 2 * L), min(hi, 2 * L)):
                t, k = t0 + j // 2, j % 2
                nc.vector.bn_stats(
                    out=bnout4[:, t, k, :], in_=xg3[gi][:, t - t0, k, :]
                )

        def emit_massage_mm1(gi):
            L, t0 = GROUPS[gi], t0s[gi]
            cols = slice(t0, t0 + L)
            means = bnout[:, cols, :, 1]
            m2s = bnout[:, cols, :, 2]
            nc.vector.tensor_reduce(
                out=stats2[:, 0, cols], in_=means, axis=Ax.X, op=Alu.add
            )
            nc.vector.tensor_tensor(
                out=mean2[:, 0:L, :], in0=means, in1=means, op=Alu.mult
            )
            nc.vector.tensor_reduce(
                out=sm2[:, 0:L], in_=mean2[:, 0:L, :], axis=Ax.X, op=Alu.add
            )
            nc.vector.tensor_reduce(
                out=sM2[:, 0:L], in_=m2s, axis=Ax.X, op=Alu.add
            )
            nc.vector.scalar_tensor_tensor(
                out=stats2[:, 1, cols], in0=sM2[:, 0:L], scalar=1.0 / 256.0,
                in1=sm2[:, 0:L], op0=Alu.mult, op1=Alu.add,
            )
            ps_mumsq = psum.tile([CS, 2, B], DT, tag=f"ps_mumsq{gi % 2}")
            nc.tensor.matmul(
                out=ps_mumsq, lhsT=sb_mask, rhs=stats2, start=True, stop=True
            )
            pend[gi] = {
                "ps_mumsq": ps_mumsq,
                "smu": smu_sbs[gi % 2],
                "svar": svar_sbs[gi % 2],
                "sc": sc_cts[gi % 2],
            }

        def emit_cp(gi):
            st = pend[gi]
            nc.scalar.activation(out=mumsq_ct, in_=st["ps_mumsq"], func=Act.Copy)
            nc.scalar.activation(
                out=muls_ct, in_=st["ps_mumsq"][:, 0, :], func=Act.Copy,
                scale=1.0 - AFWD,
            )

        def emit_s1_f(gi):
            st = pend[gi]
            smu_sb = st["smu"]
            nc.vector.tensor_tensor_scan(
                out=smu_sb[:, 1 : 1 + B], data0=sb_afwd, data1=muls_ct,
                initial=smu_sb[:, 0:1], op0=Alu.mult, op1=Alu.add,
            )
            mu_v = mumsq_ct[:, 0, :]
            msq_v = mumsq_ct[:, 1, :]
            m2g = gpool.tile([CS, B], DT, tag="m2g")
            nc.gpsimd.tensor_tensor(out=m2g, in0=mu_v, in1=mu_v, op=Alu.mult)
            var_g = gpool.tile([CS, B], DT, tag="var_g")
            nc.gpsimd.tensor_tensor(out=var_g, in0=msq_v, in1=m2g, op=Alu.subtract)
            d_g = gpool.tile([CS, B], DT, tag="d_g")
            nc.gpsimd.tensor_tensor(
                out=d_g, in0=mu_v, in1=smu_sb[:, 0:B], op=Alu.subtract
            )
            ds_g = gpool.tile([CS, B], DT, tag="ds_g")
            nc.gpsimd.tensor_tensor(out=ds_g, in0=d_g, in1=sb_sqrta, op=Alu.mult)
            d2_g = gpool.tile([CS, B], DT, tag="d2_g")
            nc.gpsimd.tensor_tensor(out=d2_g, in0=ds_g, in1=ds_g, op=Alu.mult)
            f_g = gpool.tile([CS, B], DT, tag="f_g")
            nc.gpsimd.tensor_tensor(out=f_g, in0=d2_g, in1=var_g, op=Alu.add)
            fs_g = gpool.tile([CS, B], DT, tag="fs_g")
            nc.gpsimd.tensor_tensor(out=fs_g, in0=f_g, in1=sb_oma, op=Alu.mult)
            st["fs"] = fs_g

        def emit_s2_sqrt(gi):
            st = pend[gi]
            svar_sb = st["svar"]
            nc.vector.tensor_tensor_scan(
                out=svar_sb[:, 1 : 1 + B], data0=sb_afwd, data1=st["fs"],
                initial=svar_sb[:, 0:1], op0=Alu.mult, op1=Alu.add,
            )
            nc.scalar.activation(
                out=st["sc"], in_=svar_sb[:, 0:B], func=Act.Sqrt, bias=sb_eps
            )

        def emit_rc_rb(gi):
            L, t0 = GROUPS[gi], t0s[gi]
            cols = slice(t0, t0 + L)
            st = pend[gi]
            nc.vector.reciprocal(out=rs_ct, in_=st["sc"])
            # positive smu*rs; the negation is folded into bmaskn
            nc.gpsimd.tensor_tensor(
                out=nb_ct, in0=st["smu"][:, 0:B], in1=rs_ct, op=Alu.mult
            )
            ps_rb = psum.tile([P, 2, LMAX], DT, tag="ps_rb")
            nc.tensor.matmul(
                out=ps_rb[:, 0, 0:L], lhsT=sb_bmask, rhs=rs_ct[:, cols],
                start=True, stop=True,
            )
            nc.tensor.matmul(
                out=ps_rb[:, 1, 0:L], lhsT=sb_bmaskn, rhs=nb_ct[:, cols],
                start=True, stop=True,
            )
            nc.scalar.activation(
                out=rb[:, :, cols], in_=ps_rb[:, :, 0:L], func=Act.Copy
            )

        def emit_norms_out(gi):
            L, t0 = GROUPS[gi], t0s[gi]
            cols = slice(t0, t0 + L)
            pend.pop(gi, None)
            n_dve = NORM_DVE.get(gi, 0)
            for t in range(t0, t0 + L - n_dve):
                nc.scalar.activation(
                    out=xg[gi][:, t - t0, :], in_=xg[gi][:, t - t0, :],
                    func=Act.Identity,
                    bias=rb[:, 1, t : t + 1], scale=rb[:, 0, t : t + 1],
                )
            for t in range(t0 + L - n_dve, t0 + L):
                nc.vector.tensor_scalar(
                    out=xg[gi][:, t - t0, :], in0=xg[gi][:, t - t0, :],
                    scalar1=rb[:, 0, t : t + 1], scalar2=rb[:, 1, t : t + 1],
                    op0=Alu.mult, op1=Alu.add,
                )
            nc.sync.dma_start(out=out_h[:, cols, :], in_=xg[gi])

        for s in range(NG + 2):
            a = s if s < NG else None         # group streaming in
            b = s - 1 if 1 <= s <= NG else None   # group running its chain
            c = s - 2 if s >= 2 else None     # group normalizing + storing
            if a is not None:
                emit_bn(a, 0, 2)
            if b is not None:
                emit_cp(b)
            if a is not None:
                emit_bn(a, 2, 4)
            if b is not None:
                emit_s1_f(b)
            if a is not None:
                emit_bn(a, 4, 8)
            if b is not None:
                emit_s2_sqrt(b)
            if a is not None:
                emit_bn(a, 8, 10)
            if b is not None:
                emit_rc_rb(b)
            if a is not None:
                emit_bn(a, 10, 2 * GROUPS[a])
            if c is not None:
                emit_norms_out(c)
            if a is not None:
                emit_massage_mm1(a)

    if not skip_compile:
        nc.compile()
    return nc


def _consts():
    mask = np.zeros((P, CS), np.float32)
    mask[np.arange(P), np.arange(P) % CS] = 1.0 / 16.0
    bmask = np.zeros((CS, P), np.float32)
    bmask[np.arange(P) % CS, np.arange(P)] = 1.0
    return {"mask": mask, "bmask": bmask, "bmaskn": -bmask}


def _in_map(x_shard, mu0_shard, var0_shard):
    """Build one core's input dict from its [P, B, F] shard + init vectors."""
    inits = np.stack([mu0_shard, var0_shard], axis=1).astype(np.float32)
    return {"x": x_shard, "inits": inits, **_consts()}


def kernel(**inputs):
    global LAST_EXEC_NS, LAST_RESULTS
    x = np.asarray(inputs["x"], dtype=np.float32)
    mu0 = np.asarray(inputs["mu0"], dtype=np.float32)
    var0 = np.asarray(inputs["var0"], dtype=np.float32)
    assert x.shape == (B, H, W_SP, C)

    from concourse.bass_utils import run_bass_kernel_spmd

    if "nc" not in _COMPILED:
        _COMPILED["nc"] = _build_bass()
    nc = _COMPILED["nc"]

    # [B, Q, F, C] view of x; per-core shard is [Q, CS, B, F] -> [P, B, F].
    # One global fp32->fp16 cast, then cheap fp16 transposed copies per core.
    xr = x.reshape(B, Q, F, C).astype(np.float16)
    in_maps = []
    for core in range(NCORES):
        c0 = core * CS
        xs = np.ascontiguousarray(
            xr[:, :, :, c0 : c0 + CS].transpose(1, 3, 0, 2)
        ).reshape(P, B, F)
        in_maps.append(
            _in_map(xs, mu0[c0 : c0 + CS], var0[c0 : c0 + CS])
        )

    trace = bool(int(os.environ.get("NORM_KERNEL_TRACE", "0")))
    if trace:
        _ensure_ntff_hook()
    res = run_bass_kernel_spmd(nc, in_maps, list(range(NCORES)), trace=trace)
    LAST_EXEC_NS = res.exec_time_ns
    LAST_RESULTS = res

    out = np.empty((B, Q, F, C), np.float32)
    for core in range(NCORES):
        c0 = core * CS
        o = res.results[core]["out"].reshape(Q, CS, B, F)
        out[:, :, :, c0 : c0 + CS] = o.transpose(2, 0, 3, 1)
    return out.reshape(B, H, W_SP, C)


# revision 50
# speedup vs baseline: 1.3965x; 1.0292x over previous
"""Online Normalization (forward) on 8 Trainium2 NeuronCores.

Reference semantics (per batch sample t, stats per channel over H*W):
    out_t = (x_t - s_mu_{t-1}) / sqrt(s_var_{t-1} + eps)
    mu_t  = mean(x_t);  var_t = mean(x_t^2) - mu_t^2
    s_mu_t  = a*s_mu_{t-1}  + (1-a)*mu_t
    s_var_t = a*s_var_{t-1} + (1-a)*var_t + a*(1-a)*(mu_t - s_mu_{t-1})^2

The EMA recurrence is linear, so per-sample batch stats feed small
lower-triangular matmuls on the tensor engine:
    s_mu_{t-1}  = a^t mu0  + sum_i W[i,t] mu_i,   W[i,t] = (1-a) a^{t-1-i}, i<t
    s_var_{t-1} = a^t var0 + sum_i W[i,t] f_i,    f_i = var_i + a*d_i^2,
                                                  d_i = mu_i - s_mu_{i-1}
The scan runs incrementally over tapered groups of samples so normalized
output streams out while later samples stream in.

Engine plan (v3): x lives in SBUF/HBM as fp16 (halves DMA traffic; the
correctness gate is 2e-2, fp16 quantization is ~4e-4).
  - DVE streams BN_STATS (mean+M2 per 512-elem block in one pass -- this
    replaces separate sum and square passes) plus a few small per-group
    reductions; nothing else sits in its queue except one tiny reciprocal
    per group, issued one group late so it never stalls the stream.
  - ACT streams all 32 normalizes (Identity w/ per-partition scale+bias)
    plus one small Sqrt per group.
  - Pool (gpsimd) runs the small PSUM<->SBUF copies and f-vector algebra
    of the stats chain, and triggers the output DMAs (SWDGE).
  - PE does the stats matmuls in [t, c] layout: operand-swapped combine
    (no transposes needed until the final [c, t] flip), with the mu0/var0
    init and eps folded in as extra contraction rows.

Sharding: channels C=256 split across 8 cores (32 each). Per core the
8 MiB fp16 shard is [128 partitions, 32 t, 1024 f], partition p = q*32+c
(q = one of 4 spatial blocks, c = channel).
"""

import os
import sys

import numpy as np

sys.path.insert(0, "/opt/trn_rl_repo")

B = 32          # batch (sequential scan axis)
H = 64
W_SP = 64
C = 256
NCORES = 8
CS = C // NCORES    # 32 channels per core
Q = 4               # spatial blocks per sample
F = (H * W_SP) // Q  # 1024 elements per block
P = 128             # partitions (Q*CS)
AFWD = 0.999
EPS = 1e-5
# tapered scan groups (= DMA chunk sizes, in batch samples): tiny head for
# fast pipeline fill, small tail so the final chain+normalize drains fast
GROUPS = [2, 4, 6, 8, 8, 4]
assert sum(GROUPS) == B
# normalize engine split: early groups ride ACT while DVE streams bn_stats;
# the last two groups go to DVE (free after its stream, 2x faster per pass)
NORM_DVE = {4: 8, 5: 4}   # group -> how many of its samples normalize on DVE

LAST_EXEC_NS = None
LAST_RESULTS = None
_COMPILED = {}


def _ensure_ntff_hook():
    """The axon boot degrades silently when ``antenv.axon_hooks`` is missing;
    provide the module + the ctypes-based NRT-profile hook ourselves so
    ``run_bass_kernel_spmd(trace=True)`` can capture NTFF profiles."""
    try:
        from antenv.axon_hooks import get_axon_ntff_profile_hook  # noqa: F401

        return
    except ImportError:
        pass

    import contextlib
    import ctypes
    import types

    so_path = "/opt/axon/libaxon_pjrt.so"
    state = {"hook": None}

    mod = types.ModuleType("antenv.axon_hooks")

    def set_axon_ntff_profile_hook(h):
        state["hook"] = h

    def get_axon_ntff_profile_hook():
        return state["hook"]

    mod.set_axon_ntff_profile_hook = set_axon_ntff_profile_hook
    mod.get_axon_ntff_profile_hook = get_axon_ntff_profile_hook
    import antenv

    antenv.axon_hooks = mod
    sys.modules["antenv.axon_hooks"] = mod

    if not os.path.exists(so_path):
        return
    lib = ctypes.CDLL(so_path)
    if not hasattr(lib, "axon_start_nrt_profile"):
        return
    lib.axon_start_nrt_profile.argtypes = [
        ctypes.POINTER(ctypes.c_int64),
        ctypes.c_size_t,
    ]
    lib.axon_start_nrt_profile.restype = ctypes.c_int64
    lib.axon_stop_nrt_profile.argtypes = [ctypes.c_char_p]
    lib.axon_stop_nrt_profile.restype = ctypes.c_int64

    @contextlib.contextmanager
    def _hook(output_dir, device_ids):
        import jax

        jax.devices()
        if device_ids:
            ids = (ctypes.c_int64 * len(device_ids))(*device_ids)
            rc = lib.axon_start_nrt_profile(ids, len(device_ids))
        else:
            rc = lib.axon_start_nrt_profile(None, 0)
        if rc != 0:
            raise RuntimeError(f"axon_start_nrt_profile rc={rc}")
        try:
            yield
        finally:
            n = lib.axon_stop_nrt_profile(str(output_dir).encode())
            print(f"profile: {n} file(s) written to {output_dir}", file=sys.stderr)

    state["hook"] = _hook


def _patch_fishpath():
    """The _compat FishPath shim lacks pathlib conveniences the manifest
    capture/replay helpers use."""
    import pathlib

    from concourse import _compat

    def _open(self, mode="r"):
        p = pathlib.Path(str(self))
        if "w" in mode:
            p.parent.mkdir(parents=True, exist_ok=True)
        return open(str(p), mode)

    _compat.FishPath.open = _open
    _compat.FishPath.mkdir = lambda self, **kw: pathlib.Path(str(self)).mkdir(**kw)
    _compat.FishPath.__fspath__ = lambda self: str(self)
    if not hasattr(_compat.FishPath, "parent"):
        _compat.FishPath.parent = property(
            lambda self: _compat.FishPath(pathlib.Path(str(self)).parent)
        )
    if not hasattr(_compat.FishPath, "stem"):
        _compat.FishPath.stem = property(
            lambda self: pathlib.Path(str(self)).stem
        )


def _manifest_capture_main():
    """Subprocess entry: build (schedule-only) under
    TILE_CAPTURE_MANIFEST_PATH so the schedule manifest lands on disk."""
    _patch_fishpath()
    try:
        _build_bass_raw(skip_compile=True)
    except Exception as e:  # manifest is written before trailing debug steps
        print(f"capture pass ended with: {e}", file=sys.stderr)


def _edit_manifest(path):
    """Rewrite the captured schedule order to pure issue order (sort by
    instruction number). The issue order is hand-pipelined so that every
    small cross-engine chain op sits right after the bn_stats group that
    feeds it; the CoreSim list scheduler instead floats those ops ~2 groups
    late, which serializes the whole back half of the kernel."""
    import json
    import re

    with open(path) as f:
        d = json.load(f)
    for block, order in d["order"].items():
        order.sort(key=lambda e: int(re.match(r"I-(\d+)", e["name"]).group(1)))
    with open(path, "w") as f:
        json.dump(d, f)


def _build_bass():
    # The CoreSim list scheduler handles the DMA queue interleave well; a
    # hand-ordered manifest replay was tried and regressed DMA pacing.
    return _build_bass_raw()


def _build_bass_raw(skip_compile=False):
    from contextlib import ExitStack

    import concourse.bacc as bacc
    import concourse.tile as tile
    from concourse import mybir

    DT = mybir.dt.float32
    DT16 = mybir.dt.float16
    Alu = mybir.AluOpType
    Act = mybir.ActivationFunctionType
    Ax = mybir.AxisListType

    nc = bacc.Bacc(
        "TRN2", target_bir_lowering=False, debug=False, num_devices=NCORES
    )
    x_h = nc.declare_dram_parameter("x", [P, B, F], DT16, isOutput=False)
    mask_h = nc.declare_dram_parameter("mask", [P, CS], DT, isOutput=False)
    bmask_h = nc.declare_dram_parameter("bmask", [CS, P], DT, isOutput=False)
    bmaskn_h = nc.declare_dram_parameter("bmaskn", [CS, P], DT, isOutput=False)
    inits_h = nc.declare_dram_parameter("inits", [CS, 2], DT, isOutput=False)
    out_h = nc.declare_dram_parameter("out", [P, B, F], DT16, isOutput=True)

    NG = len(GROUPS)
    LMAX = max(GROUPS)

    with tile.TileContext(nc) as tc, ExitStack() as ctx:
        consts = ctx.enter_context(tc.tile_pool(name="consts", bufs=1))
        xpool = ctx.enter_context(tc.tile_pool(name="xp", bufs=1))
        small = ctx.enter_context(tc.tile_pool(name="small", bufs=1))
        gpool = ctx.enter_context(tc.tile_pool(name="gp", bufs=2))
        psum = ctx.enter_context(tc.tile_pool(name="ps", bufs=1, space="PSUM"))

        # one tile per group: per-group input DMAs, bn_stats reads, in-place
        # normalizes, and output DMAs then carry NO false dependencies on
        # other groups' data. Trigger the first two groups' input DMAs ahead
        # of the const loads so the bn_stats stream starts as early as
        # possible; the consts are only needed ~15us in.
        xg = [
            xpool.tile([P, L, F], DT16, tag=f"xg{i}", name=f"xg{i}")
            for i, L in enumerate(GROUPS)
        ]
        xg3 = [t.rearrange("p b (two f) -> p b two f", two=2) for t in xg]
        t0s = []
        t0 = 0
        for L in GROUPS:
            t0s.append(t0)
            t0 += L
        for gi in (0, 1):
            nc.sync.dma_start(
                out=xg[gi], in_=x_h[:, t0s[gi] : t0s[gi] + GROUPS[gi], :]
            )

        sb_mask = consts.tile([P, CS], DT)       # mask[p, c] = [p%CS==c]/16
        nc.sync.dma_start(out=sb_mask, in_=mask_h[:, :])
        sb_bmask = consts.tile([CS, P], DT)      # bmask[c, p] = [p%CS==c]
        nc.sync.dma_start(out=sb_bmask, in_=bmask_h[:, :])
        sb_bmaskn = consts.tile([CS, P], DT)     # -bmask (negates nbias)
        nc.sync.dma_start(out=sb_bmaskn, in_=bmaskn_h[:, :])
        sb_sqrta = consts.tile([CS, B], DT)      # sqrt(AFWD): f = (sqrt(a)d)^2+var
        nc.vector.memset(sb_sqrta, float(AFWD ** 0.5))
        sb_afwd = consts.tile([CS, B], DT)       # scan multiplier a
        nc.vector.memset(sb_afwd, AFWD)
        sb_oma = consts.tile([CS, B], DT)        # 1-a (scales f for the var scan)
        nc.vector.memset(sb_oma, 1.0 - AFWD)
        sb_eps = consts.tile([CS, 1], DT)
        nc.vector.memset(sb_eps, EPS)

        for gi in range(2, len(GROUPS)):
            nc.sync.dma_start(
                out=xg[gi], in_=x_h[:, t0s[gi] : t0s[gi] + GROUPS[gi], :]
            )

        # bn_stats records: per sample 2 blocks x (even, odd) halves
        # = 4 records of (count, mean, M2)
        bnout = small.tile([P, B, 4, 3], DT)
        bnout4 = bnout.rearrange("p b (k two) three -> p b k (two three)", two=2)
        mean2 = small.tile([P, LMAX, 4], DT)
        sm2 = small.tile([P, LMAX], DT)
        sM2 = small.tile([P, LMAX], DT)
        # stats2[:, 0, t] = sum_x/256 per partition-block; [:, 1, t] = sum_x2/256
        stats2 = small.tile([P, 2, B], DT)
        nc.vector.memset(stats2, 0.0)

        # [c, t] layout state. The EMA recurrences run as tensor_tensor_scan
        # along the free (t) axis with fp32 internal state -- exactly the
        # reference recurrence, no W matrices and no transposes. Column 0 of
        # each scan tile holds the initial state (mu0 / var0), so columns
        # 0..B-1 of the tile ARE the "previous" states the outputs need.
        # smu/sc are double-buffered across groups (written in stage_b(g+1)
        # while stage_c(g) still reads them).
        mumsq_ct = small.tile([CS, 2, B], DT)    # raw mu / msq, ct layout
        muls_ct = small.tile([CS, B], DT)        # (1-a) * mu
        smu_sbs, svar_sbs, sc_cts = [], [], []
        for k in range(2):
            t_smu = small.tile([CS, 1 + B], DT, name=f"smu_sb{k}")
            nc.sync.dma_start(out=t_smu[:, 0:1], in_=inits_h[:, 0:1])
            smu_sbs.append(t_smu)
            t_svar = small.tile([CS, 1 + B], DT, name=f"svar_sb{k}")
            nc.sync.dma_start(out=t_svar[:, 0:1], in_=inits_h[:, 1:2])
            svar_sbs.append(t_svar)
            t_sc = small.tile([CS, B], DT, name=f"sc_ct{k}")
            sc_cts.append(t_sc)
        rs_ct = small.tile([CS, B], DT)
        nb_ct = small.tile([CS, B], DT)
        rb = small.tile([P, 2, B], DT)          # [:,0,t]=rscale, [:,1,t]=nbias

        # warm the sqrt_and_others activation table before the streaming
        # phase so no ACT_TABLE_LOAD lands mid-kernel
        warm = small.tile([1, 1], DT)
        nc.vector.memset(warm, 1.0)
        nc.scalar.activation(out=warm, in_=warm, func=Act.Sqrt)

        t0s = []
        t0 = 0
        for L in GROUPS:
            t0s.append(t0)
            t0 += L

        # Fine-grained software pipeline, replayed verbatim via the schedule
        # manifest (the CoreSim list scheduler would float the small chain
        # ops ~2 groups late, serializing the back half). Per slot s the
        # chain of group s-1 is woven BETWEEN the bn_stats of group s with
        # enough spacing that every op's cross-engine producers are done by
        # the time its engine reaches it; normalizes run two slots behind.
        pend = {}

        def emit_bn(gi, lo, hi):
            L, t0 = GROUPS[gi], t0s[gi]
            for j in range(min(lo, 2 * L), min(hi, 2 * L)):
                t, k = t0 + j // 2, j % 2
                nc.vector.bn_stats(
                    out=bnout4[:, t, k, :], in_=xg3[gi][:, t - t0, k, :]
                )

        def emit_massage_mm1(gi):
            L, t0 = GROUPS[gi], t0s[gi]
            cols = slice(t0, t0 + L)
            means = bnout[:, cols, :, 1]
            m2s = bnout[:, cols, :, 2]
            nc.vector.tensor_reduce(
                out=stats2[:, 0, cols], in_=means, axis=Ax.X, op=Alu.add
            )
            nc.vector.tensor_tensor(
                out=mean2[:, 0:L, :], in0=means, in1=means, op=Alu.mult
            )
            nc.vector.tensor_reduce(
                out=sm2[:, 0:L], in_=mean2[:, 0:L, :], axis=Ax.X, op=Alu.add
            )
            nc.vector.tensor_reduce(
                out=sM2[:, 0:L], in_=m2s, axis=Ax.X, op=Alu.add
            )
            nc.vector.scalar_tensor_tensor(
                out=stats2[:, 1, cols], in0=sM2[:, 0:L], scalar=1.0 / 256.0,
                in1=sm2[:, 0:L], op0=Alu.mult, op1=Alu.add,
            )
            ps_mumsq = psum.tile([CS, 2, B], DT, tag=f"ps_mumsq{gi % 2}")
            nc.tensor.matmul(
                out=ps_mumsq, lhsT=sb_mask, rhs=stats2, start=True, stop=True
            )
            pend[gi] = {
                "ps_mumsq": ps_mumsq,
                "smu": smu_sbs[gi % 2],
                "svar": svar_sbs[gi % 2],
                "sc": sc_cts[gi % 2],
            }

        def emit_cp(gi):
            st = pend[gi]
            nc.scalar.activation(out=mumsq_ct, in_=st["ps_mumsq"], func=Act.Copy)
            nc.scalar.activation(
                out=muls_ct, in_=st["ps_mumsq"][:, 0, :], func=Act.Copy,
                scale=1.0 - AFWD,
            )

        def emit_s1_f(gi):
            st = pend[gi]
            smu_sb = st["smu"]
            nc.vector.tensor_tensor_scan(
                out=smu_sb[:, 1 : 1 + B], data0=sb_afwd, data1=muls_ct,
                initial=smu_sb[:, 0:1], op0=Alu.mult, op1=Alu.add,
            )
            mu_v = mumsq_ct[:, 0, :]
            msq_v = mumsq_ct[:, 1, :]
            m2g = gpool.tile([CS, B], DT, tag="m2g")
            nc.gpsimd.tensor_tensor(out=m2g, in0=mu_v, in1=mu_v, op=Alu.mult)
            var_g = gpool.tile([CS, B], DT, tag="var_g")
            nc.gpsimd.tensor_tensor(out=var_g, in0=msq_v, in1=m2g, op=Alu.subtract)
            d_g = gpool.tile([CS, B], DT, tag="d_g")
            nc.gpsimd.tensor_tensor(
                out=d_g, in0=mu_v, in1=smu_sb[:, 0:B], op=Alu.subtract
            )
            ds_g = gpool.tile([CS, B], DT, tag="ds_g")
            nc.gpsimd.tensor_tensor(out=ds_g, in0=d_g, in1=sb_sqrta, op=Alu.mult)
            d2_g = gpool.tile([CS, B], DT, tag="d2_g")
            nc.gpsimd.tensor_tensor(out=d2_g, in0=ds_g, in1=ds_g, op=Alu.mult)
            f_g = gpool.tile([CS, B], DT, tag="f_g")
            nc.gpsimd.tensor_tensor(out=f_g, in0=d2_g, in1=var_g, op=Alu.add)
            fs_g = gpool.tile([CS, B], DT, tag="fs_g")
            nc.gpsimd.tensor_tensor(out=fs_g, in0=f_g, in1=sb_oma, op=Alu.mult)
            st["fs"] = fs_g

        def emit_s2_sqrt(gi):
            st = pend[gi]
            svar_sb = st["svar"]
            nc.vector.tensor_tensor_scan(
                out=svar_sb[:, 1 : 1 + B], data0=sb_afwd, data1=st["fs"],
                initial=svar_sb[:, 0:1], op0=Alu.mult, op1=Alu.add,
            )
            nc.scalar.activation(
                out=st["sc"], in_=svar_sb[:, 0:B], func=Act.Sqrt, bias=sb_eps
            )

        def emit_rc_rb(gi):
            L, t0 = GROUPS[gi], t0s[gi]
            cols = slice(t0, t0 + L)
            st = pend[gi]
            nc.vector.reciprocal(out=rs_ct, in_=st["sc"])
            # positive smu*rs; the negation is folded into bmaskn
            nc.gpsimd.tensor_tensor(
                out=nb_ct, in0=st["smu"][:, 0:B], in1=rs_ct, op=Alu.mult
            )
            ps_rb = psum.tile([P, 2, LMAX], DT, tag="ps_rb")
            nc.tensor.matmul(
                out=ps_rb[:, 0, 0:L], lhsT=sb_bmask, rhs=rs_ct[:, cols],
                start=True, stop=True,
            )
            nc.tensor.matmul(
                out=ps_rb[:, 1, 0:L], lhsT=sb_bmaskn, rhs=nb_ct[:, cols],
                start=True, stop=True,
            )
            nc.scalar.activation(
                out=rb[:, :, cols], in_=ps_rb[:, :, 0:L], func=Act.Copy
            )

        def emit_norms_out(gi):
            L, t0 = GROUPS[gi], t0s[gi]
            cols = slice(t0, t0 + L)
            pend.pop(gi, None)
            n_dve = NORM_DVE.get(gi, 0)
            for t in range(t0, t0 + L - n_dve):
                nc.scalar.activation(
                    out=xg[gi][:, t - t0, :], in_=xg[gi][:, t - t0, :],
                    func=Act.Identity,
                    bias=rb[:, 1, t : t + 1], scale=rb[:, 0, t : t + 1],
                )
            for t in range(t0 + L - n_dve, t0 + L):
                nc.vector.tensor_scalar(
                    out=xg[gi][:, t - t0, :], in0=xg[gi][:, t - t0, :],
                    scalar1=rb[:, 0, t : t + 1], scalar2=rb[:, 1, t : t + 1],
                    op0=Alu.mult, op1=Alu.add,
                )
            nc.sync.dma_start(out=out_h[:, cols, :], in_=xg[gi])

        for s in range(NG + 2):
            a = s if s < NG else None         # group streaming in
            b = s - 1 if 1 <= s <= NG else None   # group running its chain
            c = s - 2 if s >= 2 else None     # group normalizing + storing
            if a is not None:
                emit_bn(a, 0, 2)
            if b is not None:
                emit_cp(b)
            if a is not None:
                emit_bn(a, 2, 4)
            if b is not None:
                emit_s1_f(b)
            if a is not None:
                emit_bn(a, 4, 8)
            if b is not None:
                emit_s2_sqrt(b)
            if a is not None:
                emit_bn(a, 8, 10)
            if b is not None:
                emit_rc_rb(b)
            if a is not None:
                emit_bn(a, 10, 2 * GROUPS[a])
            if c is not None:
                emit_norms_out(c)
            if a is not None:
                emit_massage_mm1(a)

    if not skip_compile:
        nc.compile()
    return nc


def _consts():
    mask = np.zeros((P, CS), np.float32)
    mask[np.arange(P), np.arange(P) % CS] = 1.0 / 16.0
    bmask = np.zeros((CS, P), np.float32)
    bmask[np.arange(P) % CS, np.arange(P)] = 1.0
    return {"mask": mask, "bmask": bmask, "bmaskn": -bmask}


def _in_map(x_shard, mu0_shard, var0_shard):
    """Build one core's input dict from its [P, B, F] shard + init vectors."""
    inits = np.stack([mu0_shard, var0_shard], axis=1).astype(np.float32)
    return {"x": x_shard, "inits": inits, **_consts()}


def kernel(**inputs):
    global LAST_EXEC_NS, LAST_RESULTS
    x = np.asarray(inputs["x"], dtype=np.float32)
    mu0 = np.asarray(inputs["mu0"], dtype=np.float32)
    var0 = np.asarray(inputs["var0"], dtype=np.float32)
    assert x.shape == (B, H, W_SP, C)

    from concourse.bass_utils import run_bass_kernel_spmd

    if "nc" not in _COMPILED:
        _COMPILED["nc"] = _build_bass()
    nc = _COMPILED["nc"]

    # [B, Q, F, C] view of x; per-core shard is [Q, CS, B, F] -> [P, B, F].
    # One global fp32->fp16 cast, then cheap fp16 transposed copies per core.
    xr = x.reshape(B, Q, F, C).astype(np.float16)
    in_maps = []
    for core in range(NCORES):
        c0 = core * CS
        xs = np.ascontiguousarray(
            xr[:, :, :, c0 : c0 + CS].transpose(1, 3, 0, 2)
        ).reshape(P, B, F)
        in_maps.append(
            _in_map(xs, mu0[c0 : c0 + CS], var0[c0 : c0 + CS])
        )

    trace = bool(int(os.environ.get("NORM_KERNEL_TRACE", "0")))
    if trace:
        _ensure_ntff_hook()
    res = run_bass_kernel_spmd(nc, in_maps, list(range(NCORES)), trace=trace)
    LAST_EXEC_NS = res.exec_time_ns
    LAST_RESULTS = res

    out = np.empty((B, Q, F, C), np.float32)
    for core in range(NCORES):
        c0 = core * CS
        o = res.results[core]["out"].reshape(Q, CS, B, F)
        out[:, :, :, c0 : c0 + CS] = o.transpose(2, 0, 3, 1)
    return out.reshape(B, H, W_SP, C)


# revision 51
# speedup vs baseline: 1.5037x; 1.0768x over previous
"""Online Normalization (forward) on 8 Trainium2 NeuronCores.

Reference semantics (per batch sample t, stats per channel over H*W):
    out_t = (x_t - s_mu_{t-1}) / sqrt(s_var_{t-1} + eps)
    mu_t  = mean(x_t);  var_t = mean(x_t^2) - mu_t^2
    s_mu_t  = a*s_mu_{t-1}  + (1-a)*mu_t
    s_var_t = a*s_var_{t-1} + (1-a)*var_t + a*(1-a)*(mu_t - s_mu_{t-1})^2

The EMA recurrence is linear, so per-sample batch stats feed small
lower-triangular matmuls on the tensor engine:
    s_mu_{t-1}  = a^t mu0  + sum_i W[i,t] mu_i,   W[i,t] = (1-a) a^{t-1-i}, i<t
    s_var_{t-1} = a^t var0 + sum_i W[i,t] f_i,    f_i = var_i + a*d_i^2,
                                                  d_i = mu_i - s_mu_{i-1}
The scan runs incrementally over tapered groups of samples so normalized
output streams out while later samples stream in.

Engine plan (v3): x lives in SBUF/HBM as fp16 (halves DMA traffic; the
correctness gate is 2e-2, fp16 quantization is ~4e-4).
  - DVE streams BN_STATS (mean+M2 per 512-elem block in one pass -- this
    replaces separate sum and square passes) plus a few small per-group
    reductions; nothing else sits in its queue except one tiny reciprocal
    per group, issued one group late so it never stalls the stream.
  - ACT streams all 32 normalizes (Identity w/ per-partition scale+bias)
    plus one small Sqrt per group.
  - Pool (gpsimd) runs the small PSUM<->SBUF copies and f-vector algebra
    of the stats chain, and triggers the output DMAs (SWDGE).
  - PE does the stats matmuls in [t, c] layout: operand-swapped combine
    (no transposes needed until the final [c, t] flip), with the mu0/var0
    init and eps folded in as extra contraction rows.

Sharding: channels C=256 split across 8 cores (32 each). Per core the
8 MiB fp16 shard is [128 partitions, 32 t, 1024 f], partition p = q*32+c
(q = one of 4 spatial blocks, c = channel).
"""

import os
import sys

import numpy as np

sys.path.insert(0, "/opt/trn_rl_repo")

B = 32          # batch (sequential scan axis)
H = 64
W_SP = 64
C = 256
NCORES = 8
CS = C // NCORES    # 32 channels per core
Q = 4               # spatial blocks per sample
F = (H * W_SP) // Q  # 1024 elements per block
P = 128             # partitions (Q*CS)
AFWD = 0.999
EPS = 1e-5
# tapered scan groups (= DMA chunk sizes, in batch samples): tiny head for
# fast pipeline fill, small tail so the final chain+normalize drains fast
GROUPS = [2, 4, 6, 8, 8, 4]
assert sum(GROUPS) == B
# normalize engine split: early groups ride ACT while DVE streams bn_stats;
# the last two groups go to DVE (free after its stream, 2x faster per pass)
NORM_DVE = {4: 8, 5: 4}   # group -> how many of its samples normalize on DVE

LAST_EXEC_NS = None
LAST_RESULTS = None
_COMPILED = {}


def _ensure_ntff_hook():
    """The axon boot degrades silently when ``antenv.axon_hooks`` is missing;
    provide the module + the ctypes-based NRT-profile hook ourselves so
    ``run_bass_kernel_spmd(trace=True)`` can capture NTFF profiles."""
    try:
        from antenv.axon_hooks import get_axon_ntff_profile_hook  # noqa: F401

        return
    except ImportError:
        pass

    import contextlib
    import ctypes
    import types

    so_path = "/opt/axon/libaxon_pjrt.so"
    state = {"hook": None}

    mod = types.ModuleType("antenv.axon_hooks")

    def set_axon_ntff_profile_hook(h):
        state["hook"] = h

    def get_axon_ntff_profile_hook():
        return state["hook"]

    mod.set_axon_ntff_profile_hook = set_axon_ntff_profile_hook
    mod.get_axon_ntff_profile_hook = get_axon_ntff_profile_hook
    import antenv

    antenv.axon_hooks = mod
    sys.modules["antenv.axon_hooks"] = mod

    if not os.path.exists(so_path):
        return
    lib = ctypes.CDLL(so_path)
    if not hasattr(lib, "axon_start_nrt_profile"):
        return
    lib.axon_start_nrt_profile.argtypes = [
        ctypes.POINTER(ctypes.c_int64),
        ctypes.c_size_t,
    ]
    lib.axon_start_nrt_profile.restype = ctypes.c_int64
    lib.axon_stop_nrt_profile.argtypes = [ctypes.c_char_p]
    lib.axon_stop_nrt_profile.restype = ctypes.c_int64

    @contextlib.contextmanager
    def _hook(output_dir, device_ids):
        import jax

        jax.devices()
        if device_ids:
            ids = (ctypes.c_int64 * len(device_ids))(*device_ids)
            rc = lib.axon_start_nrt_profile(ids, len(device_ids))
        else:
            rc = lib.axon_start_nrt_profile(None, 0)
        if rc != 0:
            raise RuntimeError(f"axon_start_nrt_profile rc={rc}")
        try:
            yield
        finally:
            n = lib.axon_stop_nrt_profile(str(output_dir).encode())
            print(f"profile: {n} file(s) written to {output_dir}", file=sys.stderr)

    state["hook"] = _hook


def _patch_fishpath():
    """The _compat FishPath shim lacks pathlib conveniences the manifest
    capture/replay helpers use."""
    import pathlib

    from concourse import _compat

    def _open(self, mode="r"):
        p = pathlib.Path(str(self))
        if "w" in mode:
            p.parent.mkdir(parents=True, exist_ok=True)
        return open(str(p), mode)

    _compat.FishPath.open = _open
    _compat.FishPath.mkdir = lambda self, **kw: pathlib.Path(str(self)).mkdir(**kw)
    _compat.FishPath.__fspath__ = lambda self: str(self)
    if not hasattr(_compat.FishPath, "parent"):
        _compat.FishPath.parent = property(
            lambda self: _compat.FishPath(pathlib.Path(str(self)).parent)
        )
    if not hasattr(_compat.FishPath, "stem"):
        _compat.FishPath.stem = property(
            lambda self: pathlib.Path(str(self)).stem
        )


def _manifest_capture_main():
    """Subprocess entry: build (schedule-only) under
    TILE_CAPTURE_MANIFEST_PATH so the schedule manifest lands on disk."""
    _patch_fishpath()
    try:
        _build_bass_raw(skip_compile=True)
    except Exception as e:  # manifest is written before trailing debug steps
        print(f"capture pass ended with: {e}", file=sys.stderr)


def _edit_manifest(path):
    """Rewrite the captured schedule order to pure issue order (sort by
    instruction number). The issue order is hand-pipelined so that every
    small cross-engine chain op sits right after the bn_stats group that
    feeds it; the CoreSim list scheduler instead floats those ops ~2 groups
    late, which serializes the whole back half of the kernel."""
    import json
    import re

    with open(path) as f:
        d = json.load(f)
    for block, order in d["order"].items():
        order.sort(key=lambda e: int(re.match(r"I-(\d+)", e["name"]).group(1)))
    with open(path, "w") as f:
        json.dump(d, f)


def _build_bass():
    # The CoreSim list scheduler handles the DMA queue interleave well; a
    # hand-ordered manifest replay was tried and regressed DMA pacing.
    return _build_bass_raw()


def _build_bass_raw(skip_compile=False):
    from contextlib import ExitStack

    import concourse.bacc as bacc
    import concourse.tile as tile
    from concourse import mybir

    DT = mybir.dt.float32
    DT16 = mybir.dt.float16
    Alu = mybir.AluOpType
    Act = mybir.ActivationFunctionType
    Ax = mybir.AxisListType

    nc = bacc.Bacc(
        "TRN2", target_bir_lowering=False, debug=False, num_devices=NCORES
    )
    x_h = nc.declare_dram_parameter("x", [P, B, F], DT16, isOutput=False)
    mask_h = nc.declare_dram_parameter("mask", [P, CS], DT, isOutput=False)
    bmask_h = nc.declare_dram_parameter("bmask", [CS, P], DT, isOutput=False)
    bmaskn_h = nc.declare_dram_parameter("bmaskn", [CS, P], DT, isOutput=False)
    inits_h = nc.declare_dram_parameter("inits", [CS, 2], DT, isOutput=False)
    out_h = nc.declare_dram_parameter("out", [P, B, F], DT16, isOutput=True)

    NG = len(GROUPS)
    LMAX = max(GROUPS)

    with tile.TileContext(nc) as tc, ExitStack() as ctx:
        consts = ctx.enter_context(tc.tile_pool(name="consts", bufs=1))
        xpool = ctx.enter_context(tc.tile_pool(name="xp", bufs=1))
        small = ctx.enter_context(tc.tile_pool(name="small", bufs=1))
        gpool = ctx.enter_context(tc.tile_pool(name="gp", bufs=2))
        psum = ctx.enter_context(tc.tile_pool(name="ps", bufs=1, space="PSUM"))

        # one tile per group: per-group input DMAs, bn_stats reads, in-place
        # normalizes, and output DMAs then carry NO false dependencies on
        # other groups' data. Trigger the first two groups' input DMAs ahead
        # of the const loads so the bn_stats stream starts as early as
        # possible; the consts are only needed ~15us in.
        xg = [
            xpool.tile([P, L, F], DT16, tag=f"xg{i}", name=f"xg{i}")
            for i, L in enumerate(GROUPS)
        ]
        xg3 = [t.rearrange("p b (two f) -> p b two f", two=2) for t in xg]
        t0s = []
        t0 = 0
        for L in GROUPS:
            t0s.append(t0)
            t0 += L
        for gi in (0, 1):
            nc.sync.dma_start(
                out=xg[gi], in_=x_h[:, t0s[gi] : t0s[gi] + GROUPS[gi], :]
            )

        sb_mask = consts.tile([P, CS], DT)       # mask[p, c] = [p%CS==c]/16
        nc.sync.dma_start(out=sb_mask, in_=mask_h[:, :])
        sb_bmask = consts.tile([CS, P], DT)      # bmask[c, p] = [p%CS==c]
        nc.sync.dma_start(out=sb_bmask, in_=bmask_h[:, :])
        sb_bmaskn = consts.tile([CS, P], DT)     # -bmask (negates nbias)
        nc.sync.dma_start(out=sb_bmaskn, in_=bmaskn_h[:, :])
        sb_sqrta = consts.tile([CS, B], DT)      # sqrt(AFWD): f = (sqrt(a)d)^2+var
        nc.vector.memset(sb_sqrta, float(AFWD ** 0.5))
        sb_afwd = consts.tile([CS, B], DT)       # scan multiplier a
        nc.vector.memset(sb_afwd, AFWD)
        sb_oma = consts.tile([CS, B], DT)        # 1-a (scales f for the var scan)
        nc.vector.memset(sb_oma, 1.0 - AFWD)
        sb_eps = consts.tile([CS, 1], DT)
        nc.vector.memset(sb_eps, EPS)

        for gi in range(2, len(GROUPS)):
            nc.sync.dma_start(
                out=xg[gi], in_=x_h[:, t0s[gi] : t0s[gi] + GROUPS[gi], :]
            )

        # bn_stats records: per sample 2 blocks x (even, odd) halves
        # = 4 records of (count, mean, M2)
        bnout = small.tile([P, B, 4, 3], DT)
        bnout4 = bnout.rearrange("p b (k two) three -> p b k (two three)", two=2)
        mean2 = small.tile([P, LMAX, 4], DT)
        sm2 = small.tile([P, LMAX], DT)
        sM2 = small.tile([P, LMAX], DT)
        # stats2[:, 0, t] = sum_x/256 per partition-block; [:, 1, t] = sum_x2/256
        stats2 = small.tile([P, 2, B], DT)
        nc.vector.memset(stats2, 0.0)

        # [c, t] layout state. The EMA recurrences run as tensor_tensor_scan
        # along the free (t) axis with fp32 internal state -- exactly the
        # reference recurrence, no W matrices and no transposes. Column 0 of
        # each scan tile holds the initial state (mu0 / var0), so columns
        # 0..B-1 of the tile ARE the "previous" states the outputs need.
        # smu/sc are double-buffered across groups (written in stage_b(g+1)
        # while stage_c(g) still reads them).
        mumsq_ct = small.tile([CS, 2, B], DT)    # raw mu / msq, ct layout
        muls_ct = small.tile([CS, B], DT)        # (1-a) * mu
        smu_sbs, svar_sbs, sc_cts = [], [], []
        for k in range(2):
            t_smu = small.tile([CS, 1 + B], DT, name=f"smu_sb{k}")
            nc.sync.dma_start(out=t_smu[:, 0:1], in_=inits_h[:, 0:1])
            smu_sbs.append(t_smu)
            t_svar = small.tile([CS, 1 + B], DT, name=f"svar_sb{k}")
            nc.sync.dma_start(out=t_svar[:, 0:1], in_=inits_h[:, 1:2])
            svar_sbs.append(t_svar)
            t_sc = small.tile([CS, B], DT, name=f"sc_ct{k}")
            sc_cts.append(t_sc)
        rs_ct = small.tile([CS, B], DT)
        nb_ct = small.tile([CS, B], DT)
        rb = small.tile([P, 2, B], DT)          # [:,0,t]=rscale, [:,1,t]=nbias

        # warm the sqrt_and_others activation table before the streaming
        # phase so no ACT_TABLE_LOAD lands mid-kernel
        warm = small.tile([1, 1], DT)
        nc.vector.memset(warm, 1.0)
        nc.scalar.activation(out=warm, in_=warm, func=Act.Sqrt)

        t0s = []
        t0 = 0
        for L in GROUPS:
            t0s.append(t0)
            t0 += L

        # Fine-grained software pipeline, replayed verbatim via the schedule
        # manifest (the CoreSim list scheduler would float the small chain
        # ops ~2 groups late, serializing the back half). Per slot s the
        # chain of group s-1 is woven BETWEEN the bn_stats of group s with
        # enough spacing that every op's cross-engine producers are done by
        # the time its engine reaches it; normalizes run two slots behind.
        pend = {}

        def emit_bn(gi, lo, hi):
            L, t0 = GROUPS[gi], t0s[gi]
            for j in range(min(lo, 2 * L), min(hi, 2 * L)):
                t, k = t0 + j // 2, j % 2
                nc.vector.bn_stats(
                    out=bnout4[:, t, k, :], in_=xg3[gi][:, t - t0, k, :]
                )

        def emit_massage_mm1(gi):
            L, t0 = GROUPS[gi], t0s[gi]
            cols = slice(t0, t0 + L)
            means = bnout[:, cols, :, 1]
            m2s = bnout[:, cols, :, 2]
            nc.vector.tensor_reduce(
                out=stats2[:, 0, cols], in_=means, axis=Ax.X, op=Alu.add
            )
            nc.vector.tensor_tensor(
                out=mean2[:, 0:L, :], in0=means, in1=means, op=Alu.mult
            )
            nc.vector.tensor_reduce(
                out=sm2[:, 0:L], in_=mean2[:, 0:L, :], axis=Ax.X, op=Alu.add
            )
            nc.vector.tensor_reduce(
                out=sM2[:, 0:L], in_=m2s, axis=Ax.X, op=Alu.add
            )
            nc.vector.scalar_tensor_tensor(
                out=stats2[:, 1, cols], in0=sM2[:, 0:L], scalar=1.0 / 256.0,
                in1=sm2[:, 0:L], op0=Alu.mult, op1=Alu.add,
            )
            ps_mumsq = psum.tile([CS, 2, B], DT, tag=f"ps_mumsq{gi % 2}")
            nc.tensor.matmul(
                out=ps_mumsq, lhsT=sb_mask, rhs=stats2, start=True, stop=True
            )
            pend[gi] = {
                "ps_mumsq": ps_mumsq,
                "smu": smu_sbs[gi % 2],
                "svar": svar_sbs[gi % 2],
                "sc": sc_cts[gi % 2],
            }

        def emit_cp(gi):
            st = pend[gi]
            nc.scalar.activation(out=mumsq_ct, in_=st["ps_mumsq"], func=Act.Copy)
            nc.scalar.activation(
                out=muls_ct, in_=st["ps_mumsq"][:, 0, :], func=Act.Copy,
                scale=1.0 - AFWD,
            )

        def emit_s1_f(gi):
            st = pend[gi]
            smu_sb = st["smu"]
            nc.vector.tensor_tensor_scan(
                out=smu_sb[:, 1 : 1 + B], data0=sb_afwd, data1=muls_ct,
                initial=smu_sb[:, 0:1], op0=Alu.mult, op1=Alu.add,
            )
            mu_v = mumsq_ct[:, 0, :]
            msq_v = mumsq_ct[:, 1, :]
            m2g = gpool.tile([CS, B], DT, tag="m2g")
            nc.gpsimd.tensor_tensor(out=m2g, in0=mu_v, in1=mu_v, op=Alu.mult)
            var_g = gpool.tile([CS, B], DT, tag="var_g")
            nc.gpsimd.tensor_tensor(out=var_g, in0=msq_v, in1=m2g, op=Alu.subtract)
            d_g = gpool.tile([CS, B], DT, tag="d_g")
            nc.gpsimd.tensor_tensor(
                out=d_g, in0=mu_v, in1=smu_sb[:, 0:B], op=Alu.subtract
            )
            ds_g = gpool.tile([CS, B], DT, tag="ds_g")
            nc.gpsimd.tensor_tensor(out=ds_g, in0=d_g, in1=sb_sqrta, op=Alu.mult)
            d2_g = gpool.tile([CS, B], DT, tag="d2_g")
            nc.gpsimd.tensor_tensor(out=d2_g, in0=ds_g, in1=ds_g, op=Alu.mult)
            f_g = gpool.tile([CS, B], DT, tag="f_g")
            nc.gpsimd.tensor_tensor(out=f_g, in0=d2_g, in1=var_g, op=Alu.add)
            fs_g = gpool.tile([CS, B], DT, tag="fs_g")
            nc.gpsimd.tensor_tensor(out=fs_g, in0=f_g, in1=sb_oma, op=Alu.mult)
            st["fs"] = fs_g

        def emit_s2_sqrt(gi):
            st = pend[gi]
            svar_sb = st["svar"]
            nc.vector.tensor_tensor_scan(
                out=svar_sb[:, 1 : 1 + B], data0=sb_afwd, data1=st["fs"],
                initial=svar_sb[:, 0:1], op0=Alu.mult, op1=Alu.add,
            )
            nc.scalar.activation(
                out=st["sc"], in_=svar_sb[:, 0:B], func=Act.Sqrt, bias=sb_eps
            )

        def emit_rc_rb(gi):
            L, t0 = GROUPS[gi], t0s[gi]
            cols = slice(t0, t0 + L)
            st = pend[gi]
            nc.vector.reciprocal(out=rs_ct, in_=st["sc"])
            # positive smu*rs; the negation is folded into bmaskn
            nc.gpsimd.tensor_tensor(
                out=nb_ct, in0=st["smu"][:, 0:B], in1=rs_ct, op=Alu.mult
            )
            ps_rb = psum.tile([P, 2, LMAX], DT, tag="ps_rb")
            nc.tensor.matmul(
                out=ps_rb[:, 0, 0:L], lhsT=sb_bmask, rhs=rs_ct[:, cols],
                start=True, stop=True,
            )
            nc.tensor.matmul(
                out=ps_rb[:, 1, 0:L], lhsT=sb_bmaskn, rhs=nb_ct[:, cols],
                start=True, stop=True,
            )
            nc.scalar.activation(
                out=rb[:, :, cols], in_=ps_rb[:, :, 0:L], func=Act.Copy
            )

        def emit_norms_out(gi):
            L, t0 = GROUPS[gi], t0s[gi]
            cols = slice(t0, t0 + L)
            pend.pop(gi, None)
            n_dve = NORM_DVE.get(gi, 0)
            # per-sample output DMAs: each store streams out right behind
            # its normalize instead of waiting for the whole group
            for t in range(t0, t0 + L - n_dve):
                nc.scalar.activation(
                    out=xg[gi][:, t - t0, :], in_=xg[gi][:, t - t0, :],
                    func=Act.Identity,
                    bias=rb[:, 1, t : t + 1], scale=rb[:, 0, t : t + 1],
                )
                nc.sync.dma_start(
                    out=out_h[:, t : t + 1, :], in_=xg[gi][:, t - t0 : t - t0 + 1, :]
                )
            for t in range(t0 + L - n_dve, t0 + L):
                nc.vector.tensor_scalar(
                    out=xg[gi][:, t - t0, :], in0=xg[gi][:, t - t0, :],
                    scalar1=rb[:, 0, t : t + 1], scalar2=rb[:, 1, t : t + 1],
                    op0=Alu.mult, op1=Alu.add,
                )
                nc.sync.dma_start(
                    out=out_h[:, t : t + 1, :], in_=xg[gi][:, t - t0 : t - t0 + 1, :]
                )

        for s in range(NG + 2):
            a = s if s < NG else None         # group streaming in
            b = s - 1 if 1 <= s <= NG else None   # group running its chain
            c = s - 2 if s >= 2 else None     # group normalizing + storing
            if a is not None:
                emit_bn(a, 0, 2)
            if b is not None:
                emit_cp(b)
            if a is not None:
                emit_bn(a, 2, 4)
            if b is not None:
                emit_s1_f(b)
            if a is not None:
                emit_bn(a, 4, 8)
            if b is not None:
                emit_s2_sqrt(b)
            if a is not None:
                emit_bn(a, 8, 10)
            if b is not None:
                emit_rc_rb(b)
            if a is not None:
                emit_bn(a, 10, 2 * GROUPS[a])
            if c is not None:
                emit_norms_out(c)
            if a is not None:
                emit_massage_mm1(a)

    if not skip_compile:
        nc.compile()
    return nc


def _consts():
    mask = np.zeros((P, CS), np.float32)
    mask[np.arange(P), np.arange(P) % CS] = 1.0 / 16.0
    bmask = np.zeros((CS, P), np.float32)
    bmask[np.arange(P) % CS, np.arange(P)] = 1.0
    return {"mask": mask, "bmask": bmask, "bmaskn": -bmask}


def _in_map(x_shard, mu0_shard, var0_shard):
    """Build one core's input dict from its [P, B, F] shard + init vectors."""
    inits = np.stack([mu0_shard, var0_shard], axis=1).astype(np.float32)
    return {"x": x_shard, "inits": inits, **_consts()}


def kernel(**inputs):
    global LAST_EXEC_NS, LAST_RESULTS
    x = np.asarray(inputs["x"], dtype=np.float32)
    mu0 = np.asarray(inputs["mu0"], dtype=np.float32)
    var0 = np.asarray(inputs["var0"], dtype=np.float32)
    assert x.shape == (B, H, W_SP, C)

    from concourse.bass_utils import run_bass_kernel_spmd

    if "nc" not in _COMPILED:
        _COMPILED["nc"] = _build_bass()
    nc = _COMPILED["nc"]

    # [B, Q, F, C] view of x; per-core shard is [Q, CS, B, F] -> [P, B, F].
    # One global fp32->fp16 cast, then cheap fp16 transposed copies per core.
    xr = x.reshape(B, Q, F, C).astype(np.float16)
    in_maps = []
    for core in range(NCORES):
        c0 = core * CS
        xs = np.ascontiguousarray(
            xr[:, :, :, c0 : c0 + CS].transpose(1, 3, 0, 2)
        ).reshape(P, B, F)
        in_maps.append(
            _in_map(xs, mu0[c0 : c0 + CS], var0[c0 : c0 + CS])
        )

    trace = bool(int(os.environ.get("NORM_KERNEL_TRACE", "0")))
    if trace:
        _ensure_ntff_hook()
    res = run_bass_kernel_spmd(nc, in_maps, list(range(NCORES)), trace=trace)
    LAST_EXEC_NS = res.exec_time_ns
    LAST_RESULTS = res

    out = np.empty((B, Q, F, C), np.float32)
    for core in range(NCORES):
        c0 = core * CS
        o = res.results[core]["out"].reshape(Q, CS, B, F)
        out[:, :, :, c0 : c0 + CS] = o.transpose(2, 0, 3, 1)
    return out.reshape(B, H, W_SP, C)
